# revision 38
# baseline (speedup 1.0000x reference)
"""Trainium2 Bass kernel for Ernie4.5 attention (B=1, S=2048, HID=4096, H=32,
KVH=8, D=128), tensor-parallel over heads across 8 NeuronCores.

Core i owns q-heads 4i..4i+3, kv-head i, and wo rows [512*i, 512*(i+1)).
Each core computes its partial output [S, HID] in bf16; the host sums the 8
partials in f32.

The per-core program is ONE continuous PE stream (no phase barriers):
  1. qT/kT/vT = (w.T @ hsT-chunks), weights stationary -> [D, S] tiles; RoPE
     on qT/kT via stream_shuffle + host tables; v PE-transposed to natural.
  2. attention in scoresT layout: chunk j's blocks are WOVEN INTO projection
     piece j+1's matmul stream (scores -> exp on scalar -> mask+rowsum-acc on
     DVE -> pv deferred ~2us in PE program order so exp latency never stalls
     the PE). Rowsums via DVE accumulation + ONE ones-matmul per (h,chunk)
     instead of a ones-matmul per block (saves ~70k PE cycles).
  3. the last attention chunk (j=3) has no projection work to hide under:
     wo-projection matmuls are interleaved as fillers between its blocks
     (exp on scalar is the pipeline limit there, ~850ns/block: EXP costs
     2 cycles/elem on ACT).
  4. wo: final[sq,hid] = sum_c outT[c].T @ wo[c]; partials evicted bf16 and
     DMA'd via HWDGE (sync queue) spread across the stream, so there is no
     output-DMA tail after the last matmul.
"""

import os
import sys
from collections import deque
from contextlib import ExitStack

import numpy as np

for _p in ("/opt/trn_rl_repo",):
    if os.path.isdir(_p) and _p not in sys.path:
        sys.path.append(_p)

import ml_dtypes

import concourse.bass as bass
import concourse.mybir as mybir
import concourse.tile as tile
from concourse import bacc
from concourse.bass_utils import run_bass_kernel_spmd
from concourse.masks import make_identity

P = 128
B, S, HID, H, KVH, D = 1, 2048, 4096, 32, 8, 128
NCORES = 8
HL = H // NCORES          # 4 local q heads
NKT = HID // P            # 32 contraction tiles
NSQ = S // P              # 16 seq blocks
CW = 512                  # seq chunk width
NCH = S // CW             # 4 seq chunks
KP = 4                    # hsT k-tiles packed per DMA
WOC = 512                 # wo output chunk width
NHC = HID // WOC          # 8 wo output chunks
NCB = HL + 2              # 6 projection column blocks (4 q heads, k, v)
SCALE = float(D) ** -0.5
BASE = 10000.0

F32 = mybir.dt.float32
BF16 = mybir.dt.bfloat16
SWAP_MASK = [i ^ 1 for i in range(32)]

LAST_RESULT = None


def _build(act_dt=BF16, table_dt=F32):
    nc = bacc.Bacc("TRN2", target_bir_lowering=False, debug=False)

    hsT_d = nc.dram_tensor("hsT", [HID, S], act_dt, kind="ExternalInput").ap()
    wqkv_d = nc.dram_tensor("wqkv", [NCB, P, NKT * P], act_dt, kind="ExternalInput").ap()
    wo_d = nc.dram_tensor("wo", [HL, P, NHC, WOC], act_dt, kind="ExternalInput").ap()
    cosT_d = nc.dram_tensor("cosT", [P, S], table_dt, kind="ExternalInput").ap()
    ssinT_d = nc.dram_tensor("ssinT", [P, S], table_dt, kind="ExternalInput").ap()
    dmask_d = nc.dram_tensor("dmask", [P, CW // P, CW], act_dt, kind="ExternalInput").ap()
    out_d = nc.dram_tensor("out", [S, HID], act_dt, kind="ExternalOutput").ap()

    with tile.TileContext(nc) as tc, ExitStack() as ctx:
        const = ctx.enter_context(tc.tile_pool(name="const", bufs=1))
        wpool = ctx.enter_context(tc.tile_pool(name="wpool", bufs=1))
        tabs = ctx.enter_context(tc.tile_pool(name="tabs", bufs=1))
        res = ctx.enter_context(tc.tile_pool(name="res", bufs=1))
        hst = ctx.enter_context(tc.tile_pool(name="hst", bufs=NKT // KP + 1))
        evq = ctx.enter_context(tc.tile_pool(name="evq", bufs=2))
        rope = ctx.enter_context(tc.tile_pool(name="rope", bufs=3))
        vtmp = ctx.enter_context(tc.tile_pool(name="vtmp", bufs=2))
        probs = ctx.enter_context(tc.tile_pool(name="probs", bufs=8))
        accp = ctx.enter_context(tc.tile_pool(name="accp", bufs=3))
        norm = ctx.enter_context(tc.tile_pool(name="norm", bufs=2))
        wow = ctx.enter_context(tc.tile_pool(name="wow", bufs=12))
        # 8 bufs: each eviction waits for the DMA 8 units back to COMPLETE
        # (HBM write receipt ~2.6us); at ~860ns/unit pace 4 bufs was marginal
        outsb = ctx.enter_context(tc.tile_pool(name="outsb", bufs=8))
        # PSUM: 8 banks. psA: proj accumulation + wo pf. psS: scoresT.
        # psO: attention outT accumulators. psX: v-transpose + rowsums.
        psA = ctx.enter_context(tc.tile_pool(name="psA", bufs=3, space="PSUM"))
        psS = ctx.enter_context(tc.tile_pool(name="psS", bufs=2, space="PSUM"))
        psO = ctx.enter_context(tc.tile_pool(name="psO", bufs=2, space="PSUM"))
        psX = ctx.enter_context(tc.tile_pool(name="psX", bufs=1, space="PSUM"))

        ones_t = const.tile([P, 1], act_dt)
        nc.vector.memset(ones_t[:], 1.0)
        ident = const.tile([P, P], F32)
        make_identity(nc, ident[:])
        zbias = const.tile([P, 1], F32)
        nc.vector.memset(zbias[:], 0.0)

        w_all = wpool.tile([P, NCB, NKT * P], act_dt)

        PIECES = [(i * CW, CW) for i in range(NCH)]
        hst_tiles = {}
        _hsT_r = hsT_d.rearrange("(g kp p) s -> g p kp s", g=NKT // KP, kp=KP, p=P)

        def _load_hst_pack(p, g, split=False):
            off, width = PIECES[p]
            t = hst.tile([P, KP, CW], act_dt, tag="hst")
            if split:  # kp-granular so the first matmul starts sooner
                for kp in range(KP):
                    nc.sync.dma_start(t[:, kp, :width],
                                      _hsT_r[g, :, kp, bass.ds(off, width)])
            else:
                nc.sync.dma_start(t[:, :, :width], _hsT_r[g, :, :, bass.ds(off, width)])
            hst_tiles.setdefault(p, []).append(t)

        # one DMA per (3-group, g) slice: 16 weight DMAs instead of 48 —
        # each DMA costs ~0.6us of serialized ring time regardless of size,
        # and the clogged ring was delivering the next piece's packs late
        _wqkv_r = wqkv_d.rearrange("c p s -> p c s")

        def _load_w_trio(c0, g):
            gsl = bass.ds(g * KP * P, KP * P)
            nc.sync.dma_start(w_all[:, c0:c0 + 3, gsl], _wqkv_r[:, c0:c0 + 3, gsl])

        for g in range(NKT // KP):
            _load_w_trio(0, g)
            _load_hst_pack(0, g, split=(g == 0))
        for g in range(NKT // KP):
            _load_w_trio(3, g)

        cosT = tabs.tile([P, S], table_dt)
        nc.sync.dma_start(cosT[:], cosT_d[:, :])
        ssinT = tabs.tile([P, S], table_dt)
        nc.sync.dma_start(ssinT[:], ssinT_d[:, :])
        dmask = tabs.tile([P, CW // P, CW], act_dt)
        nc.sync.dma_start(dmask[:], dmask_d[:, :, :])

        qkT = res.tile([P, HL + 1, S], act_dt)
        v_sb = res.tile([P, NSQ, P], act_dt)
        outT = res.tile([P, HL, S], act_dt)

        # ---------------- projection helpers ----------------
        def _finish_block(p, c, ps):
            off, width = PIECES[p]
            osl = bass.ds(off, width)
            if c < HL + 1:  # q heads and k: RoPE then store
                raw = evq.tile([P, CW], act_dt, tag="raw")
                nc.scalar.copy(raw[:, :width], ps[:, :width])
                t1 = rope.tile([P, CW], act_dt, tag="t1")
                nc.vector.tensor_mul(t1[:, :width], raw[:, :width], cosT[:, osl])
                t2 = rope.tile([P, CW], act_dt, tag="t2")
                nc.vector.stream_shuffle(t2[:, :width], raw[:, :width], SWAP_MASK)
                t3 = rope.tile([P, CW], act_dt, tag="t3")
                nc.vector.tensor_mul(t3[:, :width], t2[:, :width], ssinT[:, osl])
                nc.vector.tensor_add(qkT[:, c, osl], t1[:, :width], t3[:, :width])
            else:  # v: evict then PE-transpose into natural layout
                vt = vtmp.tile([P, CW], F32, tag="vt")
                nc.scalar.copy(vt[:, :width], ps[:, :width])
                for b in range(width // P):
                    pt = psX.tile([P, P], F32, tag="x")
                    nc.tensor.transpose(pt[:], vt[:, b * P:(b + 1) * P], ident[:])
                    nc.vector.tensor_copy(v_sb[:, off // P + b, :], pt[:])

        def _load_hst_piece(p):
            for g in range(NKT // KP):
                _load_hst_pack(p, g)

        # ---------------- attention chunk emitter ----------------
        class AttnChunk:
            """Emits chunk j's attention, split into a scores side (PE matmul
            + exp + mask + rowsum-acc) and a pv side (PE matmul into po,
            deferred in PE program order so exp latency is hidden). Chunk-end
            (per head): ones-matmul rowsum + normalize chain."""

            def __init__(self, j):
                self.j = j
                self.nblk = (j + 1) * (CW // P)
                self.blocks = [(h, b) for h in range(HL) for b in range(self.nblk)]
                self.s_cursor = 0
                self.pv_cursor = 0
                self.pending = deque()  # (emit_tick, h, b, pb, o, width)
                self.po = {}
                self.acc = {}
                self.done = False

            def _osl(self, b):
                j = self.j
                t = b - j * (CW // P)
                o = t * P if t > 0 else 0
                return t, o

            def emit_scores(self, tick):
                if self.s_cursor >= len(self.blocks):
                    return False
                h, b = self.blocks[self.s_cursor]
                j = self.j
                t, o = self._osl(b)
                csl = bass.ds(j * CW + o, CW - o)
                if b == 0:
                    # bf16: DVE adds at 2x rate, and the ones-matmul streams
                    # it at 1 col/cycle (f32 moving would be half-rate)
                    self.acc[h] = accp.tile([P, CW], act_dt, tag="acc",
                                            name=f"acc_{j}_{h}")
                pss = psS.tile([P, CW], F32, tag="s")
                nc.tensor.matmul(pss[:, o:], qkT[:, HL, b * P:(b + 1) * P],
                                 qkT[:, h, csl], start=True, stop=True)
                pb = probs.tile([P, CW], act_dt, tag="pb")
                nc.scalar.activation(pb[:, o:], pss[:, o:],
                                     mybir.ActivationFunctionType.Exp,
                                     bias=zbias[:], scale=SCALE)
                if t >= 0:  # diagonal block: zero sq < sk entries
                    nc.vector.tensor_mul(pb[:, o:], pb[:, o:], dmask[:, t, o:])
                acc = self.acc[h]
                if b == 0:
                    nc.vector.tensor_copy(acc[:, :], pb[:, :])
                else:
                    nc.vector.tensor_add(acc[:, o:], acc[:, o:], pb[:, o:])
                self.pending.append((tick, h, b, pb, o))
                self.s_cursor += 1
                return True

            def emit_pv(self):
                _, h, b, pb, o = self.pending.popleft()
                if b == 0:
                    # allocated HERE (not at scores-time): the pv lag means a
                    # previous chunk's po is normalized by now, so the scores
                    # pipeline never blocks on a po bank at chunk boundaries
                    self.po[h] = psO.tile([P, CW], F32, tag="po",
                                          name=f"po_{self.j}_{h}")
                nc.tensor.matmul(self.po[h][:, o:], v_sb[:, b, :], pb[:, o:],
                                 start=(b == 0), stop=(b == self.nblk - 1))
                self.pv_cursor += 1
                if b == self.nblk - 1:
                    self._emit_h_end(h)

            def _emit_h_end(self, h):
                j = self.j
                jsl = bass.ts(j, CW)
                pr = psX.tile([1, CW], F32, tag="x")
                nc.tensor.matmul(pr[:], ones_t[:], self.acc[h][:, :],
                                 start=True, stop=True)
                # reciprocal reads PSUM directly on DVE (short queue) so the
                # pr bank frees fast -- psX has a single buf
                rc = norm.tile([1, CW], F32, tag="rc")
                sc = norm.tile([1, CW], F32, tag="sc")
                nc.vector.reciprocal_approx_accurate(rc[:], pr[:], sc[:])
                rb = norm.tile([P, CW], F32, tag="rb")
                nc.gpsimd.partition_broadcast(rb[:], rc[:], channels=P)
                nc.vector.tensor_mul(outT[:, h, jsl], self.po[h][:], rb[:])
                del self.po[h]
                del self.acc[h]
                if h == HL - 1:
                    self.done = True

            def emit_due_pvs(self, tick, lag):
                while self.pending and tick - self.pending[0][0] >= lag:
                    self.emit_pv()

            def flush(self):
                while self.s_cursor < len(self.blocks):
                    self.emit_scores(10 ** 9)
                while self.pending:
                    self.emit_pv()

        # ---------------- wo emitter ----------------
        class WoEmitter:
            """Walks hc 0..NHC-1 consuming every sq-block whose attention
            chunk has completed; revisits hcs for late chunks. One 'mm step'
            = one of the 4 head-accumulation matmuls of a (hc, sqb) unit."""

            def __init__(self):
                self.ready_sqb = []
                self.consumed = set()
                self.hc = 0
                self.sq_queue = deque()
                self.unit = None  # (hc, sqb, pf, wts, c)
                self.wts_cache = {}  # hc -> list of 4 tiles (current visit)
                self.drain_mode = False  # scalar is free once exps are done
                self.evict_flip = False
                self.units_done = 0

            def add_chunk(self, j):
                self.ready_sqb.extend(range(j * (CW // P), (j + 1) * (CW // P)))
                if self.hc >= NHC:
                    self.hc = 0  # restart pass for late-arriving chunks

            def _load_wts(self, hc):
                wts = []
                for c in range(HL):
                    wt = wow.tile([P, WOC], act_dt, tag="wt")
                    nc.sync.dma_start(wt[:], wo_d[c, :, hc, :])
                    wts.append(wt)
                self.wts_cache[hc] = wts

            def prefetch(self, hc):
                if hc is not None and hc < NHC and hc not in self.wts_cache:
                    self._load_wts(hc)

            def _peek_next_hc(self):
                """Next hc that still has unconsumed ready work (wraps for the
                second pass over late-arriving chunks)."""
                order = list(range(self.hc + 1, NHC)) + list(range(0, self.hc))
                for hc2 in order:
                    if any((hc2, s) not in self.consumed for s in self.ready_sqb):
                        return hc2
                return None

            def _next_unit(self):
                while True:
                    if self.sq_queue:
                        sqb = self.sq_queue.popleft()
                        if self.hc not in self.wts_cache:
                            self._load_wts(self.hc)
                        self.prefetch(self._peek_next_hc())
                        pf = psA.tile([P, WOC], F32, tag="a", name=f"pf_{self.hc}_{sqb}")
                        self.unit = [self.hc, sqb, pf, self.wts_cache[self.hc], 0]
                        return True
                    avail = [s for s in self.ready_sqb
                             if (self.hc, s) not in self.consumed]
                    if avail and self.hc < NHC:
                        self.sq_queue.extend(sorted(avail))
                        for s in avail:
                            self.consumed.add((self.hc, s))
                        continue
                    # move to next hc (drop its cached weights: new visit
                    # reloads if it comes back for late chunks)
                    if self.hc >= NHC:
                        return False
                    self.wts_cache.pop(self.hc, None)
                    self.hc += 1
                    if self.hc >= NHC:
                        if all((hc2, s) in self.consumed
                               for hc2 in range(NHC) for s in self.ready_sqb):
                            return False
                        self.hc = 0  # second pass for late chunks
                    continue

            def emit_mm(self):
                """Emit one matmul step; returns False when no work ready."""
                if self.unit is None:
                    if not self._next_unit():
                        return False
                hc, sqb, pf, wts, c = self.unit
                nc.tensor.matmul(pf[:], outT[:, c, sqb * P:(sqb + 1) * P],
                                 wts[c][:], start=(c == 0), stop=(c == HL - 1))
                if c == HL - 1:
                    ob = outsb.tile([P, WOC], act_dt, tag="ob")
                    # DVE PSUM-read CAST costs ~690ns vs ~450ns on scalar;
                    # scalar is exp-saturated during attn3 but idle in drain
                    if self.drain_mode and self.evict_flip:
                        nc.scalar.copy(ob[:], pf[:])
                    else:
                        nc.vector.tensor_copy(ob[:], pf[:])
                    # alternate DMA rings: the sync ring serializes transfers
                    # at ~600ns each and also carries the wt loads — one ring
                    # alone saturates at the ~860ns/unit drain pace
                    # last few units: force the low-completion-latency sync
                    # ring so the end-of-kernel barrier isn't waiting ~2us
                    # on a SWDGE receipt
                    if self.units_done >= NHC * NSQ - 5:
                        dma_eng = nc.sync
                    else:
                        dma_eng = nc.gpsimd if self.evict_flip else nc.sync
                    self.evict_flip = not self.evict_flip
                    dma_eng.dma_start(
                        out_d[sqb * P:(sqb + 1) * P, hc * WOC:(hc + 1) * WOC],
                        ob[:])
                    self.units_done += 1
                    self.unit = None
                else:
                    self.unit[4] = c + 1
                return True

            def drain(self):
                self.drain_mode = True
                while self.emit_mm():
                    pass

        wo = WoEmitter()

        # ---------------- piece 0: k-incremental projections ----------------
        def _compute_piece_kinc(p):
            off, width = PIECES[p]
            packs = hst_tiles.pop(p)
            ps0 = []
            for c in range(NCB):
                pool_c = (psA, psA, psS, psS, psO, psO)[c]
                tag_c = ("a", "a", "s", "s", "po", "po")[c]
                pc = pool_c.tile([P, CW], F32, tag=tag_c, name=f"ps0_{p}_{c}")
                ps0.append(pc)
            for ci, cs in enumerate(((0, 1, 2), (3, 4, 5))):
                for k in range(NKT):
                    for c in cs:
                        nc.tensor.matmul(
                            ps0[c][:, :width], w_all[:, c, k * P:(k + 1) * P],
                            packs[k // KP][:, k % KP, :width],
                            start=(k == 0), stop=(k == NKT - 1))
                if ci == 0:
                    # evict pass-1 groups (q heads: RoPE only, no PE work)
                    # now: frees their PSUM banks ~24us before the next
                    # piece's groups need them
                    _load_hst_piece(1)
                    for c in cs:
                        _finish_block(p, c, ps0[c])
            return ps0

        ps_p0 = _compute_piece_kinc(0)
        # v (c=5) first: its eviction+PE-transposes go to the head of the
        # scalar queue instead of waiting behind the RoPE evictions
        for c in (5, 3, 4):
            _finish_block(0, c, ps_p0[c])

        # ---------------- pieces 1..3 with attn chunk p-1 woven in ----------
        # A chunk's scores all land within its piece, but its pv/rowsum TAIL
        # spills into the next piece's slots (carried in `live`): the piece
        # boundary never stalls the PE waiting on exp chains.
        SLOTS = NCB * 8  # sub-bursts of 4 proj matmuls each
        live = []  # chunks with pending pvs from earlier pieces
        slot = 0
        for p in range(1, len(PIECES)):
            if p + 1 < len(PIECES):
                _load_hst_piece(p + 1)
            packs = hst_tiles.pop(p)
            width = PIECES[p][1]
            at = AttnChunk(p - 1)
            nblocks = len(at.blocks)
            slot0 = slot
            # v group first (see above). In the last piece, also pull k (c=4)
            # forward: attn chunk 3's first scores follow immediately at the
            # region-B boundary and need kT chunk 3 RoPE'd in time.
            corder = (5, 4, 0, 1, 2, 3) if p == len(PIECES) - 1 else (5, 0, 1, 2, 3, 4)
            for c in corder:
                ps = psA.tile([P, CW], F32, tag="a")
                for sb in range(8):
                    for k in range(sb * KP, (sb + 1) * KP):
                        nc.tensor.matmul(
                            ps[:, :width], w_all[:, c, k * P:(k + 1) * P],
                            packs[k // KP][:, k % KP, :width],
                            start=(k == 0), stop=(k == NKT - 1))
                    slot += 1
                    for old in live:
                        old.emit_due_pvs(slot, lag=2)
                    live = [o for o in live if o.pending]
                    target = (slot - slot0) * nblocks // SLOTS
                    while at.s_cursor < min(target, nblocks):
                        at.emit_scores(slot)
                    at.emit_due_pvs(slot, lag=2)
                _finish_block(p, c, ps)
            if at.pending:
                live.append(at)

        # ---------------- region B: attn chunk 3 with wo fillers ------------
        wo.add_chunk(0)
        wo.add_chunk(1)
        # chunk 2 becomes wo-ready only once its carried-over tail drains
        if not live:
            wo.add_chunk(2)
        wo.prefetch(0)
        wo.prefetch(1)
        at3 = AttnChunk(3)
        # pe-ns bookkeeping at ~2GHz: matmul of width W ~ W/2 ns
        pe_ns = 0.0
        filler_debt = 0.0
        LAGNS = 1000.0
        for i in range(len(at3.blocks)):
            _, b = at3.blocks[i]
            t, o = at3._osl(b)
            wblk = CW - o
            at3.emit_scores(pe_ns)
            pe_ns += wblk / 2.0
            # drain the chunk-2 tail carried over from piece 3 (its exps
            # finished during the final proj groups)
            if i >= 1 and live:
                for old in live:
                    for _ in range(2):
                        if old.pending:
                            old.emit_pv()
                            pe_ns += 256.0
                live = [o for o in live if o.pending]
                if not live:
                    wo.add_chunk(2)
            # exp costs ~2 cyc/elem @1.2GHz = wblk*1.667ns; PE side s+pv = wblk ns
            filler_debt += wblk * 1.667 - 2.0 * (wblk / 2.0)
            while filler_debt > 0 and i >= 1:
                if not wo.emit_mm():
                    filler_debt = 0.0
                    break
                pe_ns += WOC / 2.0
                filler_debt -= WOC / 2.0
            # release pvs whose exp is surely done (~LAGNS of PE work ago)
            while at3.pending and pe_ns - at3.pending[0][0] >= LAGNS:
                at3.emit_pv()
                pe_ns += wblk / 2.0
        # tail: release remaining pvs, filling the exp-latency with wo matmuls
        while at3.pending:
            if pe_ns - at3.pending[0][0] >= LAGNS:
                at3.emit_pv()
                pe_ns += 220.0
            elif wo.emit_mm():
                pe_ns += WOC / 2.0
            else:
                at3.emit_pv()
                pe_ns += 220.0
        wo.add_chunk(3)
        wo.drain()

    nc.compile()
    return nc


def _rope_tables():
    inv_freq = (1.0 / (BASE ** (np.arange(0, D, 2, dtype=np.float32) / D))).astype(np.float32)
    pos = np.arange(S, dtype=np.float32)[:, None]
    ang = pos * inv_freq[None, :]              # [S, D/2]
    sin = np.sin(ang).astype(np.float32).T     # [D/2, S]
    cos = np.cos(ang).astype(np.float32).T
    cosT = np.empty((D, S), np.float32)
    cosT[0::2] = cos
    cosT[1::2] = cos
    ssinT = np.empty((D, S), np.float32)
    ssinT[0::2] = -sin
    ssinT[1::2] = sin
    return cosT, ssinT


def _diag_masks():
    p = np.arange(P)[:, None, None]
    t = np.arange(CW // P)[None, :, None]
    f = np.arange(CW)[None, None, :]
    return (f >= P * t + p).astype(np.float32)


_NC_CACHE = {}


def kernel(hidden_states, wq, wk, wv, wo):
    global LAST_RESULT
    act_np = ml_dtypes.bfloat16
    key = "bf16"
    if key not in _NC_CACHE:
        _NC_CACHE[key] = _build()
    nc = _NC_CACHE[key]

    hs = np.asarray(hidden_states, np.float32).reshape(S, HID)
    hsT = np.ascontiguousarray(hs.T).astype(act_np)
    cosT, ssinT = _rope_tables()
    dmask = _diag_masks().astype(act_np)

    in_maps = []
    for i in range(NCORES):
        wqkv = np.concatenate(
            [np.asarray(wq, np.float32)[:, i * HL * D:(i + 1) * HL * D],
             np.asarray(wk, np.float32)[:, i * D:(i + 1) * D],
             np.asarray(wv, np.float32)[:, i * D:(i + 1) * D]], axis=1)
        wqkv = np.ascontiguousarray(
            wqkv.reshape(NKT, P, NCB, P).transpose(2, 1, 0, 3).reshape(NCB, P, NKT * P)
        ).astype(act_np)
        wo_i = np.ascontiguousarray(
            np.asarray(wo, np.float32)[i * HL * D:(i + 1) * HL * D, :]
            .reshape(HL, P, NHC, WOC)).astype(act_np)
        in_maps.append({
            "hsT": hsT, "wqkv": wqkv, "wo": wo_i,
            "cosT": cosT, "ssinT": ssinT, "dmask": dmask,
        })

    trace = bool(os.environ.get("BASS_KERNEL_TRACE"))
    res = run_bass_kernel_spmd(nc, in_maps, list(range(NCORES)),
                               trace=trace, trace_cores=[0] if trace else None)
    LAST_RESULT = res
    acc = np.zeros((S, HID), np.float32)
    for i in range(NCORES):
        acc += np.asarray(res.results[i]["out"], np.float32)
    return acc.reshape(B, S, HID)


# revision 40
# speedup vs baseline: 1.1856x; 1.1856x over previous
"""Trainium2 Bass kernel for Ernie4.5 attention (B=1, S=2048, HID=4096, H=32,
KVH=8, D=128), tensor-parallel over heads across 8 NeuronCores.

Core i owns q-heads 4i..4i+3, kv-head i, and wo rows [512*i, 512*(i+1)).
Each core computes its partial output [S, HID] in bf16; the host sums the 8
partials in f32.

The per-core program is ONE continuous PE stream (no phase barriers):
  1. qT/kT/vT = (w.T @ hsT-chunks), weights stationary -> [D, S] tiles; RoPE
     on qT/kT via stream_shuffle + host tables; v PE-transposed to natural.
  2. attention in scoresT layout: chunk j's blocks are WOVEN INTO projection
     piece j+1's matmul stream (scores -> exp on scalar -> mask+rowsum-acc on
     DVE -> pv deferred ~2us in PE program order so exp latency never stalls
     the PE). Rowsums via DVE accumulation + ONE ones-matmul per (h,chunk)
     instead of a ones-matmul per block (saves ~70k PE cycles).
  3. the last attention chunk (j=3) has no projection work to hide under:
     wo-projection matmuls are interleaved as fillers between its blocks
     (exp on scalar is the pipeline limit there, ~850ns/block: EXP costs
     2 cycles/elem on ACT).
  4. wo: final[sq,hid] = sum_c outT[c].T @ wo[c]; partials evicted bf16 and
     DMA'd via HWDGE (sync queue) spread across the stream, so there is no
     output-DMA tail after the last matmul.
"""

import os
import sys
from collections import deque
from contextlib import ExitStack

import numpy as np

for _p in ("/opt/trn_rl_repo",):
    if os.path.isdir(_p) and _p not in sys.path:
        sys.path.append(_p)

import ml_dtypes

import concourse.bass as bass
import concourse.mybir as mybir
import concourse.tile as tile
from concourse import bacc
from concourse.bass_utils import run_bass_kernel_spmd
from concourse.masks import make_identity

P = 128
B, S, HID, H, KVH, D = 1, 2048, 4096, 32, 8, 128
NCORES = 8
HL = H // NCORES          # 4 local q heads
NKT = HID // P            # 32 contraction tiles
NSQ = S // P              # 16 seq blocks
CW = 512                  # seq chunk width
NCH = S // CW             # 4 seq chunks
KP = 4                    # hsT k-tiles packed per DMA
WOC = 512                 # wo output chunk width
NHC = HID // WOC          # 8 wo output chunks
NCB = HL + 2              # 6 projection column blocks (4 q heads, k, v)
SCALE = float(D) ** -0.5
BASE = 10000.0

F32 = mybir.dt.float32
BF16 = mybir.dt.bfloat16
SWAP_MASK = [i ^ 1 for i in range(32)]

LAST_RESULT = None


def _build(act_dt=BF16, table_dt=BF16):
    nc = bacc.Bacc("TRN2", target_bir_lowering=False, debug=False)

    hsT_d = nc.dram_tensor("hsT", [HID, S], act_dt, kind="ExternalInput").ap()
    wqkv_d = nc.dram_tensor("wqkv", [NCB, P, NKT * P], act_dt, kind="ExternalInput").ap()
    wo_d = nc.dram_tensor("wo", [HL, P, NHC, WOC], act_dt, kind="ExternalInput").ap()
    cosT_d = nc.dram_tensor("cosT", [P, S], table_dt, kind="ExternalInput").ap()
    ssinT_d = nc.dram_tensor("ssinT", [P, S], table_dt, kind="ExternalInput").ap()
    dmask_d = nc.dram_tensor("dmask", [P, CW // P, CW], act_dt, kind="ExternalInput").ap()
    out_d = nc.dram_tensor("out", [S, HID], act_dt, kind="ExternalOutput").ap()

    with tile.TileContext(nc) as tc, ExitStack() as ctx:
        const = ctx.enter_context(tc.tile_pool(name="const", bufs=1))
        wpool = ctx.enter_context(tc.tile_pool(name="wpool", bufs=1))
        tabs = ctx.enter_context(tc.tile_pool(name="tabs", bufs=1))
        res = ctx.enter_context(tc.tile_pool(name="res", bufs=1))
        hst = ctx.enter_context(tc.tile_pool(name="hst", bufs=NKT // KP + 1))
        evq = ctx.enter_context(tc.tile_pool(name="evq", bufs=2))
        rope = ctx.enter_context(tc.tile_pool(name="rope", bufs=3))
        vtmp = ctx.enter_context(tc.tile_pool(name="vtmp", bufs=2))
        probs = ctx.enter_context(tc.tile_pool(name="probs", bufs=8))
        accp = ctx.enter_context(tc.tile_pool(name="accp", bufs=3))
        norm = ctx.enter_context(tc.tile_pool(name="norm", bufs=2))
        wow = ctx.enter_context(tc.tile_pool(name="wow", bufs=12))
        # 8 bufs: each eviction waits for the DMA 8 units back to COMPLETE
        # (HBM write receipt ~2.6us); at ~860ns/unit pace 4 bufs was marginal
        outsb = ctx.enter_context(tc.tile_pool(name="outsb", bufs=8))
        # PSUM: 8 banks. psA: proj accumulation + wo pf. psS: scoresT.
        # psO: attention outT accumulators. psX: v-transpose + rowsums.
        psA = ctx.enter_context(tc.tile_pool(name="psA", bufs=3, space="PSUM"))
        psS = ctx.enter_context(tc.tile_pool(name="psS", bufs=2, space="PSUM"))
        psO = ctx.enter_context(tc.tile_pool(name="psO", bufs=2, space="PSUM"))
        psX = ctx.enter_context(tc.tile_pool(name="psX", bufs=1, space="PSUM"))

        ones_t = const.tile([P, 1], act_dt)
        nc.vector.memset(ones_t[:], 1.0)
        ident = const.tile([P, P], F32)
        make_identity(nc, ident[:])
        zbias = const.tile([P, 1], F32)
        nc.vector.memset(zbias[:], 0.0)

        w_all = wpool.tile([P, NCB, NKT * P], act_dt)

        PIECES = [(i * CW, CW) for i in range(NCH)]
        hst_tiles = {}
        _hsT_r = hsT_d.rearrange("(g kp p) s -> g p kp s", g=NKT // KP, kp=KP, p=P)

        def _load_hst_pack(p, g, split=False):
            off, width = PIECES[p]
            t = hst.tile([P, KP, CW], act_dt, tag="hst")
            if split:  # kp-granular so the first matmul starts sooner
                for kp in range(KP):
                    nc.sync.dma_start(t[:, kp, :width],
                                      _hsT_r[g, :, kp, bass.ds(off, width)])
            else:
                nc.sync.dma_start(t[:, :, :width], _hsT_r[g, :, :, bass.ds(off, width)])
            hst_tiles.setdefault(p, []).append(t)

        # one DMA per (3-group, g) slice: 16 weight DMAs instead of 48 —
        # each DMA costs ~0.6us of serialized ring time regardless of size,
        # and the clogged ring was delivering the next piece's packs late
        _wqkv_r = wqkv_d.rearrange("c p s -> p c s")

        def _load_w_trio(c0, g):
            gsl = bass.ds(g * KP * P, KP * P)
            nc.sync.dma_start(w_all[:, c0:c0 + 3, gsl], _wqkv_r[:, c0:c0 + 3, gsl])

        for g in range(NKT // KP):
            _load_w_trio(0, g)
            _load_hst_pack(0, g, split=(g == 0))
        for g in range(NKT // KP):
            _load_w_trio(3, g)

        cosT = tabs.tile([P, S], table_dt)
        nc.sync.dma_start(cosT[:], cosT_d[:, :])
        ssinT = tabs.tile([P, S], table_dt)
        nc.sync.dma_start(ssinT[:], ssinT_d[:, :])
        dmask = tabs.tile([P, CW // P, CW], act_dt)
        nc.sync.dma_start(dmask[:], dmask_d[:, :, :])

        qkT = res.tile([P, HL + 1, S], act_dt)
        v_sb = res.tile([P, NSQ, P], act_dt)
        outT = res.tile([P, HL, S], act_dt)

        # ---------------- projection helpers ----------------
        def _finish_block(p, c, ps):
            off, width = PIECES[p]
            osl = bass.ds(off, width)
            if c < HL + 1:  # q heads and k: RoPE then store
                raw = evq.tile([P, CW], act_dt, tag="raw")
                nc.scalar.copy(raw[:, :width], ps[:, :width])
                t1 = rope.tile([P, CW], act_dt, tag="t1")
                nc.vector.tensor_mul(t1[:, :width], raw[:, :width], cosT[:, osl])
                t2 = rope.tile([P, CW], act_dt, tag="t2")
                nc.vector.stream_shuffle(t2[:, :width], raw[:, :width], SWAP_MASK)
                t3 = rope.tile([P, CW], act_dt, tag="t3")
                nc.vector.tensor_mul(t3[:, :width], t2[:, :width], ssinT[:, osl])
                nc.vector.tensor_add(qkT[:, c, osl], t1[:, :width], t3[:, :width])
            else:  # v: evict then PE-transpose into natural layout
                vt = vtmp.tile([P, CW], F32, tag="vt")
                nc.scalar.copy(vt[:, :width], ps[:, :width])
                for b in range(width // P):
                    pt = psX.tile([P, P], F32, tag="x")
                    nc.tensor.transpose(pt[:], vt[:, b * P:(b + 1) * P], ident[:])
                    nc.vector.tensor_copy(v_sb[:, off // P + b, :], pt[:])

        def _load_hst_piece(p):
            for g in range(NKT // KP):
                _load_hst_pack(p, g)

        # ---------------- attention chunk emitter ----------------
        class AttnChunk:
            """Emits chunk j's attention, split into a scores side (PE matmul
            + exp + mask + rowsum-acc) and a pv side (PE matmul into po,
            deferred in PE program order so exp latency is hidden). Chunk-end
            (per head): ones-matmul rowsum + normalize chain."""

            def __init__(self, j):
                self.j = j
                self.nblk = (j + 1) * (CW // P)
                self.blocks = [(h, b) for h in range(HL) for b in range(self.nblk)]
                self.s_cursor = 0
                self.pv_cursor = 0
                self.pending = deque()  # (emit_tick, h, b, pb, o, width)
                self.po = {}
                self.acc = {}
                self.done = False

            def _osl(self, b):
                j = self.j
                t = b - j * (CW // P)
                o = t * P if t > 0 else 0
                return t, o

            def emit_scores(self, tick):
                if self.s_cursor >= len(self.blocks):
                    return False
                h, b = self.blocks[self.s_cursor]
                j = self.j
                t, o = self._osl(b)
                csl = bass.ds(j * CW + o, CW - o)
                if b == 0:
                    # bf16: DVE adds at 2x rate, and the ones-matmul streams
                    # it at 1 col/cycle (f32 moving would be half-rate)
                    self.acc[h] = accp.tile([P, CW], act_dt, tag="acc",
                                            name=f"acc_{j}_{h}")
                pss = psS.tile([P, CW], F32, tag="s")
                nc.tensor.matmul(pss[:, o:], qkT[:, HL, b * P:(b + 1) * P],
                                 qkT[:, h, csl], start=True, stop=True)
                pb = probs.tile([P, CW], act_dt, tag="pb")
                nc.scalar.activation(pb[:, o:], pss[:, o:],
                                     mybir.ActivationFunctionType.Exp,
                                     bias=zbias[:], scale=SCALE)
                if t >= 0:  # diagonal block: zero sq < sk entries
                    nc.vector.tensor_mul(pb[:, o:], pb[:, o:], dmask[:, t, o:])
                acc = self.acc[h]
                if b == 0:
                    nc.vector.tensor_copy(acc[:, :], pb[:, :])
                else:
                    nc.vector.tensor_add(acc[:, o:], acc[:, o:], pb[:, o:])
                self.pending.append((tick, h, b, pb, o))
                self.s_cursor += 1
                return True

            def emit_pv(self):
                _, h, b, pb, o = self.pending.popleft()
                if b == 0:
                    # allocated HERE (not at scores-time): the pv lag means a
                    # previous chunk's po is normalized by now, so the scores
                    # pipeline never blocks on a po bank at chunk boundaries
                    self.po[h] = psO.tile([P, CW], F32, tag="po",
                                          name=f"po_{self.j}_{h}")
                nc.tensor.matmul(self.po[h][:, o:], v_sb[:, b, :], pb[:, o:],
                                 start=(b == 0), stop=(b == self.nblk - 1))
                self.pv_cursor += 1
                if b == self.nblk - 1:
                    self._emit_h_end(h)

            def _emit_h_end(self, h):
                j = self.j
                jsl = bass.ts(j, CW)
                pr = psX.tile([1, CW], F32, tag="x")
                nc.tensor.matmul(pr[:], ones_t[:], self.acc[h][:, :],
                                 start=True, stop=True)
                # reciprocal reads PSUM directly on DVE (short queue) so the
                # pr bank frees fast -- psX has a single buf
                rc = norm.tile([1, CW], F32, tag="rc")
                sc = norm.tile([1, CW], F32, tag="sc")
                nc.vector.reciprocal_approx_accurate(rc[:], pr[:], sc[:])
                rb = norm.tile([P, CW], F32, tag="rb")
                nc.gpsimd.partition_broadcast(rb[:], rc[:], channels=P)
                nc.vector.tensor_mul(outT[:, h, jsl], self.po[h][:], rb[:])
                del self.po[h]
                del self.acc[h]
                if h == HL - 1:
                    self.done = True

            def emit_due_pvs(self, tick, lag):
                while self.pending and tick - self.pending[0][0] >= lag:
                    self.emit_pv()

            def flush(self):
                while self.s_cursor < len(self.blocks):
                    self.emit_scores(10 ** 9)
                while self.pending:
                    self.emit_pv()

        # ---------------- wo emitter ----------------
        class WoEmitter:
            """Walks hc 0..NHC-1 consuming every sq-block whose attention
            chunk has completed; revisits hcs for late chunks. One 'mm step'
            = one of the 4 head-accumulation matmuls of a (hc, sqb) unit."""

            def __init__(self):
                self.ready_sqb = []
                self.consumed = set()
                self.hc = 0
                self.sq_queue = deque()
                self.unit = None  # (hc, sqb, pf, wts, c)
                self.wts_cache = {}  # hc -> list of 4 tiles (current visit)
                self.drain_mode = False  # scalar is free once exps are done
                self.evict_flip = False
                self.units_done = 0

            def add_chunk(self, j):
                self.ready_sqb.extend(range(j * (CW // P), (j + 1) * (CW // P)))
                if self.hc >= NHC:
                    self.hc = 0  # restart pass for late-arriving chunks

            def _load_wts(self, hc):
                wts = []
                for c in range(HL):
                    wt = wow.tile([P, WOC], act_dt, tag="wt")
                    nc.sync.dma_start(wt[:], wo_d[c, :, hc, :])
                    wts.append(wt)
                self.wts_cache[hc] = wts

            def prefetch(self, hc):
                if hc is not None and hc < NHC and hc not in self.wts_cache:
                    self._load_wts(hc)

            def _peek_next_hc(self):
                """Next hc that still has unconsumed ready work (wraps for the
                second pass over late-arriving chunks)."""
                order = list(range(self.hc + 1, NHC)) + list(range(0, self.hc))
                for hc2 in order:
                    if any((hc2, s) not in self.consumed for s in self.ready_sqb):
                        return hc2
                return None

            def _next_unit(self):
                while True:
                    if self.sq_queue:
                        sqb = self.sq_queue.popleft()
                        if self.hc not in self.wts_cache:
                            self._load_wts(self.hc)
                        self.prefetch(self._peek_next_hc())
                        pf = psA.tile([P, WOC], F32, tag="a", name=f"pf_{self.hc}_{sqb}")
                        self.unit = [self.hc, sqb, pf, self.wts_cache[self.hc], 0]
                        return True
                    avail = [s for s in self.ready_sqb
                             if (self.hc, s) not in self.consumed]
                    if avail and self.hc < NHC:
                        self.sq_queue.extend(sorted(avail))
                        for s in avail:
                            self.consumed.add((self.hc, s))
                        continue
                    # move to next hc (drop its cached weights: new visit
                    # reloads if it comes back for late chunks)
                    if self.hc >= NHC:
                        return False
                    self.wts_cache.pop(self.hc, None)
                    self.hc += 1
                    if self.hc >= NHC:
                        if all((hc2, s) in self.consumed
                               for hc2 in range(NHC) for s in self.ready_sqb):
                            return False
                        self.hc = 0  # second pass for late chunks
                    continue

            def emit_mm(self):
                """Emit one matmul step; returns False when no work ready."""
                if self.unit is None:
                    if not self._next_unit():
                        return False
                hc, sqb, pf, wts, c = self.unit
                nc.tensor.matmul(pf[:], outT[:, c, sqb * P:(sqb + 1) * P],
                                 wts[c][:], start=(c == 0), stop=(c == HL - 1))
                if c == HL - 1:
                    ob = outsb.tile([P, WOC], act_dt, tag="ob")
                    # DVE PSUM-read CAST costs ~690ns vs ~450ns on scalar;
                    # scalar is exp-saturated during attn3 but idle in drain
                    if self.drain_mode and self.evict_flip:
                        nc.scalar.copy(ob[:], pf[:])
                    else:
                        nc.vector.tensor_copy(ob[:], pf[:])
                    # alternate DMA rings: the sync ring serializes transfers
                    # at ~600ns each and also carries the wt loads — one ring
                    # alone saturates at the ~860ns/unit drain pace
                    # last few units: force the low-completion-latency sync
                    # ring so the end-of-kernel barrier isn't waiting ~2us
                    # on a SWDGE receipt
                    if self.units_done >= NHC * NSQ - 5:
                        dma_eng = nc.sync
                    else:
                        dma_eng = nc.gpsimd if self.evict_flip else nc.sync
                    self.evict_flip = not self.evict_flip
                    dma_eng.dma_start(
                        out_d[sqb * P:(sqb + 1) * P, hc * WOC:(hc + 1) * WOC],
                        ob[:])
                    self.units_done += 1
                    self.unit = None
                else:
                    self.unit[4] = c + 1
                return True

            def drain(self):
                self.drain_mode = True
                while self.emit_mm():
                    pass

        wo = WoEmitter()

        # ---------------- piece 0: k-incremental projections ----------------
        def _compute_piece_kinc(p):
            off, width = PIECES[p]
            packs = hst_tiles.pop(p)
            ps0 = []
            for c in range(NCB):
                pool_c = (psA, psA, psS, psS, psO, psO)[c]
                tag_c = ("a", "a", "s", "s", "po", "po")[c]
                pc = pool_c.tile([P, CW], F32, tag=tag_c, name=f"ps0_{p}_{c}")
                ps0.append(pc)
            for ci, cs in enumerate(((0, 1, 2), (3, 4, 5))):
                for k in range(NKT):
                    for c in cs:
                        nc.tensor.matmul(
                            ps0[c][:, :width], w_all[:, c, k * P:(k + 1) * P],
                            packs[k // KP][:, k % KP, :width],
                            start=(k == 0), stop=(k == NKT - 1))
                if ci == 0:
                    # evict pass-1 groups (q heads: RoPE only, no PE work)
                    # now: frees their PSUM banks ~24us before the next
                    # piece's groups need them
                    _load_hst_piece(1)
                    for c in cs:
                        _finish_block(p, c, ps0[c])
            return ps0

        ps_p0 = _compute_piece_kinc(0)
        # v (c=5) first: its eviction+PE-transposes go to the head of the
        # scalar queue instead of waiting behind the RoPE evictions
        for c in (5, 3, 4):
            _finish_block(0, c, ps_p0[c])

        # ---------------- pieces 1..3 with attn chunk p-1 woven in ----------
        # A chunk's scores all land within its piece, but its pv/rowsum TAIL
        # spills into the next piece's slots (carried in `live`): the piece
        # boundary never stalls the PE waiting on exp chains.
        SLOTS = NCB * 8  # sub-bursts of 4 proj matmuls each
        live = []  # chunks with pending pvs from earlier pieces
        slot = 0
        for p in range(1, len(PIECES)):
            if p + 1 < len(PIECES):
                _load_hst_piece(p + 1)
            packs = hst_tiles.pop(p)
            width = PIECES[p][1]
            at = AttnChunk(p - 1)
            nblocks = len(at.blocks)
            slot0 = slot
            # v group first (see above). In the last piece, also pull k (c=4)
            # forward: attn chunk 3's first scores follow immediately at the
            # region-B boundary and need kT chunk 3 RoPE'd in time.
            corder = (5, 4, 0, 1, 2, 3) if p == len(PIECES) - 1 else (5, 0, 1, 2, 3, 4)
            for c in corder:
                ps = psA.tile([P, CW], F32, tag="a")
                for sb in range(8):
                    for k in range(sb * KP, (sb + 1) * KP):
                        nc.tensor.matmul(
                            ps[:, :width], w_all[:, c, k * P:(k + 1) * P],
                            packs[k // KP][:, k % KP, :width],
                            start=(k == 0), stop=(k == NKT - 1))
                    slot += 1
                    for old in live:
                        old.emit_due_pvs(slot, lag=2)
                    live = [o for o in live if o.pending]
                    target = (slot - slot0) * nblocks // SLOTS
                    while at.s_cursor < min(target, nblocks):
                        at.emit_scores(slot)
                    at.emit_due_pvs(slot, lag=2)
                _finish_block(p, c, ps)
            if at.pending:
                live.append(at)

        # ---------------- region B: attn chunk 3 with wo fillers ------------
        wo.add_chunk(0)
        wo.add_chunk(1)
        # chunk 2 becomes wo-ready only once its carried-over tail drains
        if not live:
            wo.add_chunk(2)
        wo.prefetch(0)
        wo.prefetch(1)
        at3 = AttnChunk(3)
        # pe-ns bookkeeping at ~2GHz: matmul of width W ~ W/2 ns
        pe_ns = 0.0
        filler_debt = 0.0
        LAGNS = 1000.0
        for i in range(len(at3.blocks)):
            _, b = at3.blocks[i]
            t, o = at3._osl(b)
            wblk = CW - o
            at3.emit_scores(pe_ns)
            pe_ns += wblk / 2.0
            # drain the chunk-2 tail carried over from piece 3 (its exps
            # finished during the final proj groups)
            if i >= 1 and live:
                for old in live:
                    for _ in range(2):
                        if old.pending:
                            old.emit_pv()
                            pe_ns += 256.0
                live = [o for o in live if o.pending]
                if not live:
                    wo.add_chunk(2)
            # exp costs ~2 cyc/elem @1.2GHz = wblk*1.667ns; PE side s+pv = wblk ns
            filler_debt += wblk * 1.667 - 2.0 * (wblk / 2.0)
            while filler_debt > 0 and i >= 1:
                if not wo.emit_mm():
                    filler_debt = 0.0
                    break
                pe_ns += WOC / 2.0
                filler_debt -= WOC / 2.0
            # release pvs whose exp is surely done (~LAGNS of PE work ago)
            while at3.pending and pe_ns - at3.pending[0][0] >= LAGNS:
                at3.emit_pv()
                pe_ns += wblk / 2.0
        # tail: release remaining pvs, filling the exp-latency with wo matmuls
        while at3.pending:
            if pe_ns - at3.pending[0][0] >= LAGNS:
                at3.emit_pv()
                pe_ns += 220.0
            elif wo.emit_mm():
                pe_ns += WOC / 2.0
            else:
                at3.emit_pv()
                pe_ns += 220.0
        wo.add_chunk(3)
        wo.drain()

    nc.compile()
    return nc


def _rope_tables():
    inv_freq = (1.0 / (BASE ** (np.arange(0, D, 2, dtype=np.float32) / D))).astype(np.float32)
    pos = np.arange(S, dtype=np.float32)[:, None]
    ang = pos * inv_freq[None, :]              # [S, D/2]
    sin = np.sin(ang).astype(np.float32).T     # [D/2, S]
    cos = np.cos(ang).astype(np.float32).T
    cosT = np.empty((D, S), np.float32)
    cosT[0::2] = cos
    cosT[1::2] = cos
    ssinT = np.empty((D, S), np.float32)
    ssinT[0::2] = -sin
    ssinT[1::2] = sin
    return cosT, ssinT


def _diag_masks():
    p = np.arange(P)[:, None, None]
    t = np.arange(CW // P)[None, :, None]
    f = np.arange(CW)[None, None, :]
    return (f >= P * t + p).astype(np.float32)


_NC_CACHE = {}


def kernel(hidden_states, wq, wk, wv, wo):
    global LAST_RESULT
    act_np = ml_dtypes.bfloat16
    key = "bf16"
    if key not in _NC_CACHE:
        _NC_CACHE[key] = _build()
    nc = _NC_CACHE[key]

    hs = np.asarray(hidden_states, np.float32).reshape(S, HID)
    hsT = np.ascontiguousarray(hs.T).astype(act_np)
    # bf16 tables: halves their DMA and keeps the RoPE muls all-bf16 on DVE
    cosT, ssinT = (t.astype(act_np) for t in _rope_tables())
    dmask = _diag_masks().astype(act_np)

    in_maps = []
    for i in range(NCORES):
        wqkv = np.concatenate(
            [np.asarray(wq, np.float32)[:, i * HL * D:(i + 1) * HL * D],
             np.asarray(wk, np.float32)[:, i * D:(i + 1) * D],
             np.asarray(wv, np.float32)[:, i * D:(i + 1) * D]], axis=1)
        wqkv = np.ascontiguousarray(
            wqkv.reshape(NKT, P, NCB, P).transpose(2, 1, 0, 3).reshape(NCB, P, NKT * P)
        ).astype(act_np)
        wo_i = np.ascontiguousarray(
            np.asarray(wo, np.float32)[i * HL * D:(i + 1) * HL * D, :]
            .reshape(HL, P, NHC, WOC)).astype(act_np)
        in_maps.append({
            "hsT": hsT, "wqkv": wqkv, "wo": wo_i,
            "cosT": cosT, "ssinT": ssinT, "dmask": dmask,
        })

    trace = bool(os.environ.get("BASS_KERNEL_TRACE"))
    res = run_bass_kernel_spmd(nc, in_maps, list(range(NCORES)),
                               trace=trace, trace_cores=[0] if trace else None)
    LAST_RESULT = res
    acc = np.zeros((S, HID), np.float32)
    for i in range(NCORES):
        acc += np.asarray(res.results[i]["out"], np.float32)
    return acc.reshape(B, S, HID)


# revision 42
# speedup vs baseline: 1.1882x; 1.0022x over previous
"""Trainium2 Bass kernel for Ernie4.5 attention (B=1, S=2048, HID=4096, H=32,
KVH=8, D=128), tensor-parallel over heads across 8 NeuronCores.

Core i owns q-heads 4i..4i+3, kv-head i, and wo rows [512*i, 512*(i+1)).
Each core computes its partial output [S, HID] in bf16; the host sums the 8
partials in f32.

The per-core program is ONE continuous PE stream (no phase barriers):
  1. qT/kT/vT = (w.T @ hsT-chunks), weights stationary -> [D, S] tiles; RoPE
     on qT/kT via stream_shuffle + host tables; v PE-transposed to natural.
  2. attention in scoresT layout: chunk j's blocks are WOVEN INTO projection
     piece j+1's matmul stream (scores -> exp on scalar -> mask+rowsum-acc on
     DVE -> pv deferred ~2us in PE program order so exp latency never stalls
     the PE). Rowsums via DVE accumulation + ONE ones-matmul per (h,chunk)
     instead of a ones-matmul per block (saves ~70k PE cycles).
  3. the last attention chunk (j=3) has no projection work to hide under:
     wo-projection matmuls are interleaved as fillers between its blocks
     (exp on scalar is the pipeline limit there, ~850ns/block: EXP costs
     2 cycles/elem on ACT).
  4. wo: final[sq,hid] = sum_c outT[c].T @ wo[c]; partials evicted bf16 and
     DMA'd via HWDGE (sync queue) spread across the stream, so there is no
     output-DMA tail after the last matmul.
"""

import os
import sys
from collections import deque
from contextlib import ExitStack

import numpy as np

for _p in ("/opt/trn_rl_repo",):
    if os.path.isdir(_p) and _p not in sys.path:
        sys.path.append(_p)

import ml_dtypes

import concourse.bass as bass
import concourse.mybir as mybir
import concourse.tile as tile
from concourse import bacc
from concourse.bass_utils import run_bass_kernel_spmd
from concourse.masks import make_identity

P = 128
B, S, HID, H, KVH, D = 1, 2048, 4096, 32, 8, 128
NCORES = 8
HL = H // NCORES          # 4 local q heads
NKT = HID // P            # 32 contraction tiles
NSQ = S // P              # 16 seq blocks
CW = 512                  # seq chunk width
NCH = S // CW             # 4 seq chunks
KP = 4                    # hsT k-tiles packed per DMA
WOC = 512                 # wo output chunk width
NHC = HID // WOC          # 8 wo output chunks
NCB = HL + 2              # 6 projection column blocks (4 q heads, k, v)
SCALE = float(D) ** -0.5
BASE = 10000.0

F32 = mybir.dt.float32
BF16 = mybir.dt.bfloat16
SWAP_MASK = [i ^ 1 for i in range(32)]

LAST_RESULT = None


def _build(act_dt=BF16, table_dt=BF16):
    nc = bacc.Bacc("TRN2", target_bir_lowering=False, debug=False)

    hsT_d = nc.dram_tensor("hsT", [HID, S], act_dt, kind="ExternalInput").ap()
    wqkv_d = nc.dram_tensor("wqkv", [NCB, P, NKT * P], act_dt, kind="ExternalInput").ap()
    wo_d = nc.dram_tensor("wo", [HL, P, NHC, WOC], act_dt, kind="ExternalInput").ap()
    cosT_d = nc.dram_tensor("cosT", [P, S], table_dt, kind="ExternalInput").ap()
    ssinT_d = nc.dram_tensor("ssinT", [P, S], table_dt, kind="ExternalInput").ap()
    dmask_d = nc.dram_tensor("dmask", [P, CW // P, CW], act_dt, kind="ExternalInput").ap()
    out_d = nc.dram_tensor("out", [S, HID], act_dt, kind="ExternalOutput").ap()

    with tile.TileContext(nc) as tc, ExitStack() as ctx:
        const = ctx.enter_context(tc.tile_pool(name="const", bufs=1))
        wpool = ctx.enter_context(tc.tile_pool(name="wpool", bufs=1))
        tabs = ctx.enter_context(tc.tile_pool(name="tabs", bufs=1))
        res = ctx.enter_context(tc.tile_pool(name="res", bufs=1))
        hst = ctx.enter_context(tc.tile_pool(name="hst", bufs=NKT // KP + 1))
        evq = ctx.enter_context(tc.tile_pool(name="evq", bufs=2))
        rope = ctx.enter_context(tc.tile_pool(name="rope", bufs=3))
        vtmp = ctx.enter_context(tc.tile_pool(name="vtmp", bufs=2))
        probs = ctx.enter_context(tc.tile_pool(name="probs", bufs=8))
        accp = ctx.enter_context(tc.tile_pool(name="accp", bufs=3))
        norm = ctx.enter_context(tc.tile_pool(name="norm", bufs=2))
        wow = ctx.enter_context(tc.tile_pool(name="wow", bufs=12))
        # 8 bufs: each eviction waits for the DMA 8 units back to COMPLETE
        # (HBM write receipt ~2.6us); at ~860ns/unit pace 4 bufs was marginal
        outsb = ctx.enter_context(tc.tile_pool(name="outsb", bufs=8))
        # PSUM: 8 banks. psA: proj accumulation + wo pf. psS: scoresT.
        # psO: attention outT accumulators. psX: v-transpose + rowsums.
        psA = ctx.enter_context(tc.tile_pool(name="psA", bufs=3, space="PSUM"))
        psS = ctx.enter_context(tc.tile_pool(name="psS", bufs=2, space="PSUM"))
        psO = ctx.enter_context(tc.tile_pool(name="psO", bufs=2, space="PSUM"))
        psX = ctx.enter_context(tc.tile_pool(name="psX", bufs=1, space="PSUM"))

        ones_t = const.tile([P, 1], act_dt)
        nc.vector.memset(ones_t[:], 1.0)
        ident = const.tile([P, P], F32)
        make_identity(nc, ident[:])
        zbias = const.tile([P, 1], F32)
        nc.vector.memset(zbias[:], 0.0)

        w_all = wpool.tile([P, NCB, NKT * P], act_dt)

        PIECES = [(i * CW, CW) for i in range(NCH)]
        hst_tiles = {}
        _hsT_r = hsT_d.rearrange("(g kp p) s -> g p kp s", g=NKT // KP, kp=KP, p=P)

        def _load_hst_pack(p, g, split=False):
            off, width = PIECES[p]
            t = hst.tile([P, KP, CW], act_dt, tag="hst")
            if split:  # kp-granular so the first matmul starts sooner
                for kp in range(KP):
                    nc.sync.dma_start(t[:, kp, :width],
                                      _hsT_r[g, :, kp, bass.ds(off, width)])
            else:
                nc.sync.dma_start(t[:, :, :width], _hsT_r[g, :, :, bass.ds(off, width)])
            hst_tiles.setdefault(p, []).append(t)

        # one DMA per (3-group, g) slice: 16 weight DMAs instead of 48 —
        # each DMA costs ~0.6us of serialized ring time regardless of size,
        # and the clogged ring was delivering the next piece's packs late
        _wqkv_r = wqkv_d.rearrange("c p s -> p c s")

        def _load_w_trio(c0, g):
            gsl = bass.ds(g * KP * P, KP * P)
            nc.sync.dma_start(w_all[:, c0:c0 + 3, gsl], _wqkv_r[:, c0:c0 + 3, gsl])

        for g in range(NKT // KP):
            _load_w_trio(0, g)
            _load_hst_pack(0, g, split=(g == 0))
        for g in range(NKT // KP):
            _load_w_trio(3, g)

        cosT = tabs.tile([P, S], table_dt)
        nc.sync.dma_start(cosT[:], cosT_d[:, :])
        ssinT = tabs.tile([P, S], table_dt)
        nc.sync.dma_start(ssinT[:], ssinT_d[:, :])
        dmask = tabs.tile([P, CW // P, CW], act_dt)
        nc.sync.dma_start(dmask[:], dmask_d[:, :, :])

        qkT = res.tile([P, HL + 1, S], act_dt)
        v_sb = res.tile([P, NSQ, P], act_dt)
        outT = res.tile([P, HL, S], act_dt)

        # ---------------- projection helpers ----------------
        def _finish_block(p, c, ps):
            off, width = PIECES[p]
            osl = bass.ds(off, width)
            if c < HL + 1:  # q heads and k: RoPE then store
                raw = evq.tile([P, CW], act_dt, tag="raw")
                nc.scalar.copy(raw[:, :width], ps[:, :width])
                t1 = rope.tile([P, CW], act_dt, tag="t1")
                nc.vector.tensor_mul(t1[:, :width], raw[:, :width], cosT[:, osl])
                t2 = rope.tile([P, CW], act_dt, tag="t2")
                nc.vector.stream_shuffle(t2[:, :width], raw[:, :width], SWAP_MASK)
                t3 = rope.tile([P, CW], act_dt, tag="t3")
                nc.vector.tensor_mul(t3[:, :width], t2[:, :width], ssinT[:, osl])
                nc.vector.tensor_add(qkT[:, c, osl], t1[:, :width], t3[:, :width])
            else:  # v: evict then PE-transpose into natural layout
                vt = vtmp.tile([P, CW], F32, tag="vt")
                nc.scalar.copy(vt[:, :width], ps[:, :width])
                for b in range(width // P):
                    pt = psX.tile([P, P], F32, tag="x")
                    nc.tensor.transpose(pt[:], vt[:, b * P:(b + 1) * P], ident[:])
                    nc.vector.tensor_copy(v_sb[:, off // P + b, :], pt[:])

        def _load_hst_piece(p):
            for g in range(NKT // KP):
                _load_hst_pack(p, g)

        # ---------------- attention chunk emitter ----------------
        class AttnChunk:
            """Emits chunk j's attention, split into a scores side (PE matmul
            + exp + mask + rowsum-acc) and a pv side (PE matmul into po,
            deferred in PE program order so exp latency is hidden). Chunk-end
            (per head): ones-matmul rowsum + normalize chain."""

            def __init__(self, j):
                self.j = j
                self.nblk = (j + 1) * (CW // P)
                self.blocks = [(h, b) for h in range(HL) for b in range(self.nblk)]
                self.s_cursor = 0
                self.pv_cursor = 0
                self.pending = deque()  # (emit_tick, h, b, pb, o, width)
                self.po = {}
                self.acc = {}
                self.norm_pending = None
                self.done = False

            def _osl(self, b):
                j = self.j
                t = b - j * (CW // P)
                o = t * P if t > 0 else 0
                return t, o

            def emit_scores(self, tick):
                if self.s_cursor >= len(self.blocks):
                    return False
                h, b = self.blocks[self.s_cursor]
                j = self.j
                t, o = self._osl(b)
                csl = bass.ds(j * CW + o, CW - o)
                if b == 0:
                    # bf16: DVE adds at 2x rate, and the ones-matmul streams
                    # it at 1 col/cycle (f32 moving would be half-rate)
                    self.acc[h] = accp.tile([P, CW], act_dt, tag="acc",
                                            name=f"acc_{j}_{h}")
                pss = psS.tile([P, CW], F32, tag="s")
                nc.tensor.matmul(pss[:, o:], qkT[:, HL, b * P:(b + 1) * P],
                                 qkT[:, h, csl], start=True, stop=True)
                pb = probs.tile([P, CW], act_dt, tag="pb")
                nc.scalar.activation(pb[:, o:], pss[:, o:],
                                     mybir.ActivationFunctionType.Exp,
                                     bias=zbias[:], scale=SCALE)
                if t >= 0:  # diagonal block: zero sq < sk entries
                    nc.vector.tensor_mul(pb[:, o:], pb[:, o:], dmask[:, t, o:])
                acc = self.acc[h]
                if b == 0:
                    nc.vector.tensor_copy(acc[:, :], pb[:, :])
                else:
                    nc.vector.tensor_add(acc[:, o:], acc[:, o:], pb[:, o:])
                self.pending.append((tick, h, b, pb, o))
                self.s_cursor += 1
                return True

            def emit_pv(self):
                _, h, b, pb, o = self.pending.popleft()
                if b == 0:
                    # allocated HERE (not at scores-time): the pv lag means a
                    # previous chunk's po is normalized by now, so the scores
                    # pipeline never blocks on a po bank at chunk boundaries
                    self.po[h] = psO.tile([P, CW], F32, tag="po",
                                          name=f"po_{self.j}_{h}")
                nc.tensor.matmul(self.po[h][:, o:], v_sb[:, b, :], pb[:, o:],
                                 start=(b == 0), stop=(b == self.nblk - 1))
                self.pv_cursor += 1
                if b == self.nblk - 1:
                    self._emit_h_end(h)

            def _emit_h_end(self, h):
                j = self.j
                jsl = bass.ts(j, CW)
                # the previous head's norm-mul goes out NOW: its broadcast
                # input is long done, so it can't sit at the DVE queue head
                # (waiting on gpsimd) in front of masks/adds the next pvs need
                if self.norm_pending is not None:
                    self.norm_pending()
                    self.norm_pending = None
                pr = psX.tile([1, CW], F32, tag="x")
                nc.tensor.matmul(pr[:], ones_t[:], self.acc[h][:, :],
                                 start=True, stop=True)
                # reciprocal reads PSUM directly on DVE (short queue) so the
                # pr bank frees fast -- psX has a single buf
                rc = norm.tile([1, CW], F32, tag="rc")
                sc = norm.tile([1, CW], F32, tag="sc")
                nc.vector.reciprocal_approx_accurate(rc[:], pr[:], sc[:])
                rb = norm.tile([P, CW], F32, tag="rb")
                nc.gpsimd.partition_broadcast(rb[:], rc[:], channels=P)
                po = self.po.pop(h)
                del self.acc[h]

                def _mul(po=po, rb=rb, h=h, jsl=jsl):
                    nc.vector.tensor_mul(outT[:, h, jsl], po[:], rb[:])

                if h == HL - 1:
                    _mul()  # chunk end: nothing left in-chunk to block
                    self.done = True
                else:
                    self.norm_pending = _mul

            def emit_due_pvs(self, tick, lag):
                while self.pending and tick - self.pending[0][0] >= lag:
                    self.emit_pv()

            def flush(self):
                while self.s_cursor < len(self.blocks):
                    self.emit_scores(10 ** 9)
                while self.pending:
                    self.emit_pv()

        # ---------------- wo emitter ----------------
        class WoEmitter:
            """Walks hc 0..NHC-1 consuming every sq-block whose attention
            chunk has completed; revisits hcs for late chunks. One 'mm step'
            = one of the 4 head-accumulation matmuls of a (hc, sqb) unit."""

            def __init__(self):
                self.ready_sqb = []
                self.consumed = set()
                self.hc = 0
                self.sq_queue = deque()
                self.unit = None  # (hc, sqb, pf, wts, c)
                self.wts_cache = {}  # hc -> list of 4 tiles (current visit)
                self.drain_mode = False  # scalar is free once exps are done
                self.evict_flip = False
                self.units_done = 0

            def add_chunk(self, j):
                self.ready_sqb.extend(range(j * (CW // P), (j + 1) * (CW // P)))
                if self.hc >= NHC:
                    self.hc = 0  # restart pass for late-arriving chunks

            def _load_wts(self, hc):
                wts = []
                for c in range(HL):
                    wt = wow.tile([P, WOC], act_dt, tag="wt")
                    nc.sync.dma_start(wt[:], wo_d[c, :, hc, :])
                    wts.append(wt)
                self.wts_cache[hc] = wts

            def prefetch(self, hc):
                if hc is not None and hc < NHC and hc not in self.wts_cache:
                    self._load_wts(hc)

            def _peek_next_hc(self):
                """Next hc that still has unconsumed ready work (wraps for the
                second pass over late-arriving chunks)."""
                order = list(range(self.hc + 1, NHC)) + list(range(0, self.hc))
                for hc2 in order:
                    if any((hc2, s) not in self.consumed for s in self.ready_sqb):
                        return hc2
                return None

            def _next_unit(self):
                while True:
                    if self.sq_queue:
                        sqb = self.sq_queue.popleft()
                        if self.hc not in self.wts_cache:
                            self._load_wts(self.hc)
                        self.prefetch(self._peek_next_hc())
                        pf = psA.tile([P, WOC], F32, tag="a", name=f"pf_{self.hc}_{sqb}")
                        self.unit = [self.hc, sqb, pf, self.wts_cache[self.hc], 0]
                        return True
                    avail = [s for s in self.ready_sqb
                             if (self.hc, s) not in self.consumed]
                    if avail and self.hc < NHC:
                        self.sq_queue.extend(sorted(avail))
                        for s in avail:
                            self.consumed.add((self.hc, s))
                        continue
                    # move to next hc (drop its cached weights: new visit
                    # reloads if it comes back for late chunks)
                    if self.hc >= NHC:
                        return False
                    self.wts_cache.pop(self.hc, None)
                    self.hc += 1
                    if self.hc >= NHC:
                        if all((hc2, s) in self.consumed
                               for hc2 in range(NHC) for s in self.ready_sqb):
                            return False
                        self.hc = 0  # second pass for late chunks
                    continue

            def emit_mm(self):
                """Emit one matmul step; returns False when no work ready."""
                if self.unit is None:
                    if not self._next_unit():
                        return False
                hc, sqb, pf, wts, c = self.unit
                nc.tensor.matmul(pf[:], outT[:, c, sqb * P:(sqb + 1) * P],
                                 wts[c][:], start=(c == 0), stop=(c == HL - 1))
                if c == HL - 1:
                    ob = outsb.tile([P, WOC], act_dt, tag="ob")
                    # DVE PSUM-read CAST costs ~690ns vs ~450ns on scalar;
                    # scalar is exp-saturated during attn3 but idle in drain
                    if self.drain_mode and self.evict_flip:
                        nc.scalar.copy(ob[:], pf[:])
                    else:
                        nc.vector.tensor_copy(ob[:], pf[:])
                    # alternate DMA rings: the sync ring serializes transfers
                    # at ~600ns each and also carries the wt loads — one ring
                    # alone saturates at the ~860ns/unit drain pace
                    # last few units: force the low-completion-latency sync
                    # ring so the end-of-kernel barrier isn't waiting ~2us
                    # on a SWDGE receipt
                    if self.units_done >= NHC * NSQ - 5:
                        dma_eng = nc.sync
                    else:
                        dma_eng = nc.gpsimd if self.evict_flip else nc.sync
                    self.evict_flip = not self.evict_flip
                    dma_eng.dma_start(
                        out_d[sqb * P:(sqb + 1) * P, hc * WOC:(hc + 1) * WOC],
                        ob[:])
                    self.units_done += 1
                    self.unit = None
                else:
                    self.unit[4] = c + 1
                return True

            def drain(self):
                self.drain_mode = True
                while self.emit_mm():
                    pass

        wo = WoEmitter()

        # ---------------- piece 0: k-incremental projections ----------------
        def _compute_piece_kinc(p):
            off, width = PIECES[p]
            packs = hst_tiles.pop(p)
            ps0 = []
            for c in range(NCB):
                pool_c = (psA, psA, psS, psS, psO, psO)[c]
                tag_c = ("a", "a", "s", "s", "po", "po")[c]
                pc = pool_c.tile([P, CW], F32, tag=tag_c, name=f"ps0_{p}_{c}")
                ps0.append(pc)
            for ci, cs in enumerate(((0, 1, 2), (3, 4, 5))):
                for k in range(NKT):
                    for c in cs:
                        nc.tensor.matmul(
                            ps0[c][:, :width], w_all[:, c, k * P:(k + 1) * P],
                            packs[k // KP][:, k % KP, :width],
                            start=(k == 0), stop=(k == NKT - 1))
                if ci == 0:
                    # evict pass-1 groups (q heads: RoPE only, no PE work)
                    # now: frees their PSUM banks ~24us before the next
                    # piece's groups need them
                    _load_hst_piece(1)
                    for c in cs:
                        _finish_block(p, c, ps0[c])
            return ps0

        ps_p0 = _compute_piece_kinc(0)
        # v (c=5) first: its eviction+PE-transposes go to the head of the
        # scalar queue instead of waiting behind the RoPE evictions
        for c in (5, 3, 4):
            _finish_block(0, c, ps_p0[c])

        # ---------------- pieces 1..3 with attn chunk p-1 woven in ----------
        # A chunk's scores all land within its piece, but its pv/rowsum TAIL
        # spills into the next piece's slots (carried in `live`): the piece
        # boundary never stalls the PE waiting on exp chains.
        SLOTS = NCB * 8  # sub-bursts of 4 proj matmuls each
        live = []  # chunks with pending pvs from earlier pieces
        slot = 0
        for p in range(1, len(PIECES)):
            if p + 1 < len(PIECES):
                _load_hst_piece(p + 1)
            packs = hst_tiles.pop(p)
            width = PIECES[p][1]
            at = AttnChunk(p - 1)
            nblocks = len(at.blocks)
            slot0 = slot
            # v group first (see above). In the last piece, also pull k (c=4)
            # forward: attn chunk 3's first scores follow immediately at the
            # region-B boundary and need kT chunk 3 RoPE'd in time.
            corder = (5, 4, 0, 1, 2, 3) if p == len(PIECES) - 1 else (5, 0, 1, 2, 3, 4)
            for c in corder:
                ps = psA.tile([P, CW], F32, tag="a")
                for sb in range(8):
                    for k in range(sb * KP, (sb + 1) * KP):
                        nc.tensor.matmul(
                            ps[:, :width], w_all[:, c, k * P:(k + 1) * P],
                            packs[k // KP][:, k % KP, :width],
                            start=(k == 0), stop=(k == NKT - 1))
                    slot += 1
                    for old in live:
                        old.emit_due_pvs(slot, lag=2)
                    live = [o for o in live if o.pending]
                    target = (slot - slot0) * nblocks // SLOTS
                    while at.s_cursor < min(target, nblocks):
                        at.emit_scores(slot)
                    at.emit_due_pvs(slot, lag=2)
                _finish_block(p, c, ps)
            if at.pending:
                live.append(at)

        # ---------------- region B: attn chunk 3 with wo fillers ------------
        wo.add_chunk(0)
        wo.add_chunk(1)
        # chunk 2 becomes wo-ready only once its carried-over tail drains
        if not live:
            wo.add_chunk(2)
        wo.prefetch(0)
        wo.prefetch(1)
        at3 = AttnChunk(3)
        # pe-ns bookkeeping at ~2GHz: matmul of width W ~ W/2 ns
        pe_ns = 0.0
        filler_debt = 0.0
        LAGNS = 1000.0
        for i in range(len(at3.blocks)):
            _, b = at3.blocks[i]
            t, o = at3._osl(b)
            wblk = CW - o
            at3.emit_scores(pe_ns)
            pe_ns += wblk / 2.0
            # drain the chunk-2 tail carried over from piece 3 (its exps
            # finished during the final proj groups)
            if i >= 1 and live:
                for old in live:
                    for _ in range(2):
                        if old.pending:
                            old.emit_pv()
                            pe_ns += 256.0
                live = [o for o in live if o.pending]
                if not live:
                    wo.add_chunk(2)
            # exp costs ~2 cyc/elem @1.2GHz = wblk*1.667ns; PE side s+pv = wblk ns
            filler_debt += wblk * 1.667 - 2.0 * (wblk / 2.0)
            while filler_debt > 0 and i >= 1:
                if not wo.emit_mm():
                    filler_debt = 0.0
                    break
                pe_ns += WOC / 2.0
                filler_debt -= WOC / 2.0
            # release pvs whose exp is surely done (~LAGNS of PE work ago)
            while at3.pending and pe_ns - at3.pending[0][0] >= LAGNS:
                at3.emit_pv()
                pe_ns += wblk / 2.0
        # tail: release remaining pvs, filling the exp-latency with wo matmuls
        while at3.pending:
            if pe_ns - at3.pending[0][0] >= LAGNS:
                at3.emit_pv()
                pe_ns += 220.0
            elif wo.emit_mm():
                pe_ns += WOC / 2.0
            else:
                at3.emit_pv()
                pe_ns += 220.0
        wo.add_chunk(3)
        wo.drain()

    nc.compile()
    return nc


def _rope_tables():
    inv_freq = (1.0 / (BASE ** (np.arange(0, D, 2, dtype=np.float32) / D))).astype(np.float32)
    pos = np.arange(S, dtype=np.float32)[:, None]
    ang = pos * inv_freq[None, :]              # [S, D/2]
    sin = np.sin(ang).astype(np.float32).T     # [D/2, S]
    cos = np.cos(ang).astype(np.float32).T
    cosT = np.empty((D, S), np.float32)
    cosT[0::2] = cos
    cosT[1::2] = cos
    ssinT = np.empty((D, S), np.float32)
    ssinT[0::2] = -sin
    ssinT[1::2] = sin
    return cosT, ssinT


def _diag_masks():
    p = np.arange(P)[:, None, None]
    t = np.arange(CW // P)[None, :, None]
    f = np.arange(CW)[None, None, :]
    return (f >= P * t + p).astype(np.float32)


_NC_CACHE = {}


def kernel(hidden_states, wq, wk, wv, wo):
    global LAST_RESULT
    act_np = ml_dtypes.bfloat16
    key = "bf16"
    if key not in _NC_CACHE:
        _NC_CACHE[key] = _build()
    nc = _NC_CACHE[key]

    hs = np.asarray(hidden_states, np.float32).reshape(S, HID)
    hsT = np.ascontiguousarray(hs.T).astype(act_np)
    # bf16 tables: halves their DMA and keeps the RoPE muls all-bf16 on DVE
    cosT, ssinT = (t.astype(act_np) for t in _rope_tables())
    dmask = _diag_masks().astype(act_np)

    in_maps = []
    for i in range(NCORES):
        wqkv = np.concatenate(
            [np.asarray(wq, np.float32)[:, i * HL * D:(i + 1) * HL * D],
             np.asarray(wk, np.float32)[:, i * D:(i + 1) * D],
             np.asarray(wv, np.float32)[:, i * D:(i + 1) * D]], axis=1)
        wqkv = np.ascontiguousarray(
            wqkv.reshape(NKT, P, NCB, P).transpose(2, 1, 0, 3).reshape(NCB, P, NKT * P)
        ).astype(act_np)
        wo_i = np.ascontiguousarray(
            np.asarray(wo, np.float32)[i * HL * D:(i + 1) * HL * D, :]
            .reshape(HL, P, NHC, WOC)).astype(act_np)
        in_maps.append({
            "hsT": hsT, "wqkv": wqkv, "wo": wo_i,
            "cosT": cosT, "ssinT": ssinT, "dmask": dmask,
        })

    trace = bool(os.environ.get("BASS_KERNEL_TRACE"))
    res = run_bass_kernel_spmd(nc, in_maps, list(range(NCORES)),
                               trace=trace, trace_cores=[0] if trace else None)
    LAST_RESULT = res
    acc = np.zeros((S, HID), np.float32)
    for i in range(NCORES):
        acc += np.asarray(res.results[i]["out"], np.float32)
    return acc.reshape(B, S, HID)


# revision 43
# speedup vs baseline: 1.1902x; 1.0017x over previous
"""Trainium2 Bass kernel for Ernie4.5 attention (B=1, S=2048, HID=4096, H=32,
KVH=8, D=128), tensor-parallel over heads across 8 NeuronCores.

Core i owns q-heads 4i..4i+3, kv-head i, and wo rows [512*i, 512*(i+1)).
Each core computes its partial output [S, HID] in bf16; the host sums the 8
partials in f32.

The per-core program is ONE continuous PE stream (no phase barriers):
  1. qT/kT/vT = (w.T @ hsT-chunks), weights stationary -> [D, S] tiles; RoPE
     on qT/kT via stream_shuffle + host tables; v PE-transposed to natural.
  2. attention in scoresT layout: chunk j's blocks are WOVEN INTO projection
     piece j+1's matmul stream (scores -> exp on scalar -> mask+rowsum-acc on
     DVE -> pv deferred ~2us in PE program order so exp latency never stalls
     the PE). Rowsums via DVE accumulation + ONE ones-matmul per (h,chunk)
     instead of a ones-matmul per block (saves ~70k PE cycles).
  3. the last attention chunk (j=3) has no projection work to hide under:
     wo-projection matmuls are interleaved as fillers between its blocks
     (exp on scalar is the pipeline limit there, ~850ns/block: EXP costs
     2 cycles/elem on ACT).
  4. wo: final[sq,hid] = sum_c outT[c].T @ wo[c]; partials evicted bf16 and
     DMA'd via HWDGE (sync queue) spread across the stream, so there is no
     output-DMA tail after the last matmul.
"""

import os
import sys
from collections import deque
from contextlib import ExitStack

import numpy as np

for _p in ("/opt/trn_rl_repo",):
    if os.path.isdir(_p) and _p not in sys.path:
        sys.path.append(_p)

import ml_dtypes

import concourse.bass as bass
import concourse.mybir as mybir
import concourse.tile as tile
from concourse import bacc
from concourse.bass_utils import run_bass_kernel_spmd
from concourse.masks import make_identity

P = 128
B, S, HID, H, KVH, D = 1, 2048, 4096, 32, 8, 128
NCORES = 8
HL = H // NCORES          # 4 local q heads
NKT = HID // P            # 32 contraction tiles
NSQ = S // P              # 16 seq blocks
CW = 512                  # seq chunk width
NCH = S // CW             # 4 seq chunks
KP = 4                    # hsT k-tiles packed per DMA
WOC = 512                 # wo output chunk width
NHC = HID // WOC          # 8 wo output chunks
NCB = HL + 2              # 6 projection column blocks (4 q heads, k, v)
SCALE = float(D) ** -0.5
BASE = 10000.0

F32 = mybir.dt.float32
BF16 = mybir.dt.bfloat16
SWAP_MASK = [i ^ 1 for i in range(32)]

LAST_RESULT = None


def _build(act_dt=BF16, table_dt=BF16):
    nc = bacc.Bacc("TRN2", target_bir_lowering=False, debug=False)

    hsT_d = nc.dram_tensor("hsT", [HID, S], act_dt, kind="ExternalInput").ap()
    wqkv_d = nc.dram_tensor("wqkv", [NCB, P, NKT * P], act_dt, kind="ExternalInput").ap()
    wo_d = nc.dram_tensor("wo", [HL, P, NHC, WOC], act_dt, kind="ExternalInput").ap()
    cosT_d = nc.dram_tensor("cosT", [P, S], table_dt, kind="ExternalInput").ap()
    ssinT_d = nc.dram_tensor("ssinT", [P, S], table_dt, kind="ExternalInput").ap()
    dmask_d = nc.dram_tensor("dmask", [P, CW // P, CW], act_dt, kind="ExternalInput").ap()
    out_d = nc.dram_tensor("out", [S, HID], act_dt, kind="ExternalOutput").ap()

    with tile.TileContext(nc) as tc, ExitStack() as ctx:
        const = ctx.enter_context(tc.tile_pool(name="const", bufs=1))
        wpool = ctx.enter_context(tc.tile_pool(name="wpool", bufs=1))
        tabs = ctx.enter_context(tc.tile_pool(name="tabs", bufs=1))
        res = ctx.enter_context(tc.tile_pool(name="res", bufs=1))
        hst = ctx.enter_context(tc.tile_pool(name="hst", bufs=NKT // KP + 1))
        evq = ctx.enter_context(tc.tile_pool(name="evq", bufs=2))
        rope = ctx.enter_context(tc.tile_pool(name="rope", bufs=3))
        vtmp = ctx.enter_context(tc.tile_pool(name="vtmp", bufs=2))
        probs = ctx.enter_context(tc.tile_pool(name="probs", bufs=8))
        accp = ctx.enter_context(tc.tile_pool(name="accp", bufs=3))
        norm = ctx.enter_context(tc.tile_pool(name="norm", bufs=2))
        wow = ctx.enter_context(tc.tile_pool(name="wow", bufs=12))
        # 8 bufs: each eviction waits for the DMA 8 units back to COMPLETE
        # (HBM write receipt ~2.6us); at ~860ns/unit pace 4 bufs was marginal
        outsb = ctx.enter_context(tc.tile_pool(name="outsb", bufs=8))
        # PSUM: 8 banks. psA: proj accumulation + wo pf. psS: scoresT.
        # psO: attention outT accumulators. psX: v-transpose + rowsums.
        psA = ctx.enter_context(tc.tile_pool(name="psA", bufs=3, space="PSUM"))
        psS = ctx.enter_context(tc.tile_pool(name="psS", bufs=2, space="PSUM"))
        psO = ctx.enter_context(tc.tile_pool(name="psO", bufs=2, space="PSUM"))
        psX = ctx.enter_context(tc.tile_pool(name="psX", bufs=1, space="PSUM"))

        ones_t = const.tile([P, 1], act_dt)
        nc.vector.memset(ones_t[:], 1.0)
        ident = const.tile([P, P], F32)
        make_identity(nc, ident[:])
        zbias = const.tile([P, 1], F32)
        nc.vector.memset(zbias[:], 0.0)

        # HAM warm-up: the PE idles ~4us waiting for the first weight DMA,
        # and the first ~13 real matmuls would run at K=4/8 (half clock).
        # Burn the DMA wait with dummy matmuls on on-chip data so the HAM
        # activity window fires before real work arrives.
        warm_sb = const.tile([P, P], act_dt)
        nc.vector.memset(warm_sb[:], 0.0)
        warm_ps = psX.tile([P, P], F32, tag="x")
        for _ in range(28):
            nc.tensor.matmul(warm_ps[:], warm_sb[:], warm_sb[:],
                             start=True, stop=True)

        w_all = wpool.tile([P, NCB, NKT * P], act_dt)

        PIECES = [(i * CW, CW) for i in range(NCH)]
        hst_tiles = {}
        _hsT_r = hsT_d.rearrange("(g kp p) s -> g p kp s", g=NKT // KP, kp=KP, p=P)

        def _load_hst_pack(p, g, split=False):
            off, width = PIECES[p]
            t = hst.tile([P, KP, CW], act_dt, tag="hst")
            if split:  # kp-granular so the first matmul starts sooner
                for kp in range(KP):
                    nc.sync.dma_start(t[:, kp, :width],
                                      _hsT_r[g, :, kp, bass.ds(off, width)])
            else:
                nc.sync.dma_start(t[:, :, :width], _hsT_r[g, :, :, bass.ds(off, width)])
            hst_tiles.setdefault(p, []).append(t)

        # one DMA per (3-group, g) slice: 16 weight DMAs instead of 48 —
        # each DMA costs ~0.6us of serialized ring time regardless of size,
        # and the clogged ring was delivering the next piece's packs late
        _wqkv_r = wqkv_d.rearrange("c p s -> p c s")

        def _load_w_trio(c0, g):
            gsl = bass.ds(g * KP * P, KP * P)
            nc.sync.dma_start(w_all[:, c0:c0 + 3, gsl], _wqkv_r[:, c0:c0 + 3, gsl])

        for g in range(NKT // KP):
            _load_w_trio(0, g)
            _load_hst_pack(0, g, split=(g == 0))
        for g in range(NKT // KP):
            _load_w_trio(3, g)

        cosT = tabs.tile([P, S], table_dt)
        nc.sync.dma_start(cosT[:], cosT_d[:, :])
        ssinT = tabs.tile([P, S], table_dt)
        nc.sync.dma_start(ssinT[:], ssinT_d[:, :])
        dmask = tabs.tile([P, CW // P, CW], act_dt)
        nc.sync.dma_start(dmask[:], dmask_d[:, :, :])

        qkT = res.tile([P, HL + 1, S], act_dt)
        v_sb = res.tile([P, NSQ, P], act_dt)
        outT = res.tile([P, HL, S], act_dt)

        # ---------------- projection helpers ----------------
        def _finish_block(p, c, ps):
            off, width = PIECES[p]
            osl = bass.ds(off, width)
            if c < HL + 1:  # q heads and k: RoPE then store
                raw = evq.tile([P, CW], act_dt, tag="raw")
                nc.scalar.copy(raw[:, :width], ps[:, :width])
                t1 = rope.tile([P, CW], act_dt, tag="t1")
                nc.vector.tensor_mul(t1[:, :width], raw[:, :width], cosT[:, osl])
                t2 = rope.tile([P, CW], act_dt, tag="t2")
                nc.vector.stream_shuffle(t2[:, :width], raw[:, :width], SWAP_MASK)
                t3 = rope.tile([P, CW], act_dt, tag="t3")
                nc.vector.tensor_mul(t3[:, :width], t2[:, :width], ssinT[:, osl])
                nc.vector.tensor_add(qkT[:, c, osl], t1[:, :width], t3[:, :width])
            else:  # v: evict then PE-transpose into natural layout
                vt = vtmp.tile([P, CW], F32, tag="vt")
                nc.scalar.copy(vt[:, :width], ps[:, :width])
                for b in range(width // P):
                    pt = psX.tile([P, P], F32, tag="x")
                    nc.tensor.transpose(pt[:], vt[:, b * P:(b + 1) * P], ident[:])
                    nc.vector.tensor_copy(v_sb[:, off // P + b, :], pt[:])

        def _load_hst_piece(p):
            for g in range(NKT // KP):
                _load_hst_pack(p, g)

        # ---------------- attention chunk emitter ----------------
        class AttnChunk:
            """Emits chunk j's attention, split into a scores side (PE matmul
            + exp + mask + rowsum-acc) and a pv side (PE matmul into po,
            deferred in PE program order so exp latency is hidden). Chunk-end
            (per head): ones-matmul rowsum + normalize chain."""

            def __init__(self, j):
                self.j = j
                self.nblk = (j + 1) * (CW // P)
                self.blocks = [(h, b) for h in range(HL) for b in range(self.nblk)]
                self.s_cursor = 0
                self.pv_cursor = 0
                self.pending = deque()  # (emit_tick, h, b, pb, o, width)
                self.po = {}
                self.acc = {}
                self.norm_pending = None
                self.done = False

            def _osl(self, b):
                j = self.j
                t = b - j * (CW // P)
                o = t * P if t > 0 else 0
                return t, o

            def emit_scores(self, tick):
                if self.s_cursor >= len(self.blocks):
                    return False
                h, b = self.blocks[self.s_cursor]
                j = self.j
                t, o = self._osl(b)
                csl = bass.ds(j * CW + o, CW - o)
                if b == 0:
                    # bf16: DVE adds at 2x rate, and the ones-matmul streams
                    # it at 1 col/cycle (f32 moving would be half-rate)
                    self.acc[h] = accp.tile([P, CW], act_dt, tag="acc",
                                            name=f"acc_{j}_{h}")
                pss = psS.tile([P, CW], F32, tag="s")
                nc.tensor.matmul(pss[:, o:], qkT[:, HL, b * P:(b + 1) * P],
                                 qkT[:, h, csl], start=True, stop=True)
                pb = probs.tile([P, CW], act_dt, tag="pb")
                nc.scalar.activation(pb[:, o:], pss[:, o:],
                                     mybir.ActivationFunctionType.Exp,
                                     bias=zbias[:], scale=SCALE)
                if t >= 0:  # diagonal block: zero sq < sk entries
                    nc.vector.tensor_mul(pb[:, o:], pb[:, o:], dmask[:, t, o:])
                acc = self.acc[h]
                if b == 0:
                    nc.vector.tensor_copy(acc[:, :], pb[:, :])
                else:
                    nc.vector.tensor_add(acc[:, o:], acc[:, o:], pb[:, o:])
                self.pending.append((tick, h, b, pb, o))
                self.s_cursor += 1
                return True

            def emit_pv(self):
                _, h, b, pb, o = self.pending.popleft()
                if b == 0:
                    # allocated HERE (not at scores-time): the pv lag means a
                    # previous chunk's po is normalized by now, so the scores
                    # pipeline never blocks on a po bank at chunk boundaries
                    self.po[h] = psO.tile([P, CW], F32, tag="po",
                                          name=f"po_{self.j}_{h}")
                nc.tensor.matmul(self.po[h][:, o:], v_sb[:, b, :], pb[:, o:],
                                 start=(b == 0), stop=(b == self.nblk - 1))
                self.pv_cursor += 1
                if b == self.nblk - 1:
                    self._emit_h_end(h)

            def _emit_h_end(self, h):
                j = self.j
                jsl = bass.ts(j, CW)
                # the previous head's norm-mul goes out NOW: its broadcast
                # input is long done, so it can't sit at the DVE queue head
                # (waiting on gpsimd) in front of masks/adds the next pvs need
                if self.norm_pending is not None:
                    self.norm_pending()
                    self.norm_pending = None
                pr = psX.tile([1, CW], F32, tag="x")
                nc.tensor.matmul(pr[:], ones_t[:], self.acc[h][:, :],
                                 start=True, stop=True)
                # reciprocal reads PSUM directly on DVE (short queue) so the
                # pr bank frees fast -- psX has a single buf
                rc = norm.tile([1, CW], F32, tag="rc")
                sc = norm.tile([1, CW], F32, tag="sc")
                nc.vector.reciprocal_approx_accurate(rc[:], pr[:], sc[:])
                rb = norm.tile([P, CW], F32, tag="rb")
                nc.gpsimd.partition_broadcast(rb[:], rc[:], channels=P)
                po = self.po.pop(h)
                del self.acc[h]

                def _mul(po=po, rb=rb, h=h, jsl=jsl):
                    nc.vector.tensor_mul(outT[:, h, jsl], po[:], rb[:])

                if h == HL - 1:
                    _mul()  # chunk end: nothing left in-chunk to block
                    self.done = True
                else:
                    self.norm_pending = _mul

            def emit_due_pvs(self, tick, lag):
                while self.pending and tick - self.pending[0][0] >= lag:
                    self.emit_pv()

            def flush(self):
                while self.s_cursor < len(self.blocks):
                    self.emit_scores(10 ** 9)
                while self.pending:
                    self.emit_pv()

        # ---------------- wo emitter ----------------
        class WoEmitter:
            """Walks hc 0..NHC-1 consuming every sq-block whose attention
            chunk has completed; revisits hcs for late chunks. One 'mm step'
            = one of the 4 head-accumulation matmuls of a (hc, sqb) unit."""

            def __init__(self):
                self.ready_sqb = []
                self.consumed = set()
                self.hc = 0
                self.sq_queue = deque()
                self.unit = None  # (hc, sqb, pf, wts, c)
                self.wts_cache = {}  # hc -> list of 4 tiles (current visit)
                self.drain_mode = False  # scalar is free once exps are done
                self.evict_flip = False
                self.units_done = 0

            def add_chunk(self, j):
                self.ready_sqb.extend(range(j * (CW // P), (j + 1) * (CW // P)))
                if self.hc >= NHC:
                    self.hc = 0  # restart pass for late-arriving chunks

            def _load_wts(self, hc):
                wts = []
                for c in range(HL):
                    wt = wow.tile([P, WOC], act_dt, tag="wt")
                    nc.sync.dma_start(wt[:], wo_d[c, :, hc, :])
                    wts.append(wt)
                self.wts_cache[hc] = wts

            def prefetch(self, hc):
                if hc is not None and hc < NHC and hc not in self.wts_cache:
                    self._load_wts(hc)

            def _peek_next_hc(self):
                """Next hc that still has unconsumed ready work (wraps for the
                second pass over late-arriving chunks)."""
                order = list(range(self.hc + 1, NHC)) + list(range(0, self.hc))
                for hc2 in order:
                    if any((hc2, s) not in self.consumed for s in self.ready_sqb):
                        return hc2
                return None

            def _next_unit(self):
                while True:
                    if self.sq_queue:
                        sqb = self.sq_queue.popleft()
                        if self.hc not in self.wts_cache:
                            self._load_wts(self.hc)
                        self.prefetch(self._peek_next_hc())
                        pf = psA.tile([P, WOC], F32, tag="a", name=f"pf_{self.hc}_{sqb}")
                        self.unit = [self.hc, sqb, pf, self.wts_cache[self.hc], 0]
                        return True
                    avail = [s for s in self.ready_sqb
                             if (self.hc, s) not in self.consumed]
                    if avail and self.hc < NHC:
                        self.sq_queue.extend(sorted(avail))
                        for s in avail:
                            self.consumed.add((self.hc, s))
                        continue
                    # move to next hc (drop its cached weights: new visit
                    # reloads if it comes back for late chunks)
                    if self.hc >= NHC:
                        return False
                    self.wts_cache.pop(self.hc, None)
                    self.hc += 1
                    if self.hc >= NHC:
                        if all((hc2, s) in self.consumed
                               for hc2 in range(NHC) for s in self.ready_sqb):
                            return False
                        self.hc = 0  # second pass for late chunks
                    continue

            def emit_mm(self):
                """Emit one matmul step; returns False when no work ready."""
                if self.unit is None:
                    if not self._next_unit():
                        return False
                hc, sqb, pf, wts, c = self.unit
                nc.tensor.matmul(pf[:], outT[:, c, sqb * P:(sqb + 1) * P],
                                 wts[c][:], start=(c == 0), stop=(c == HL - 1))
                if c == HL - 1:
                    ob = outsb.tile([P, WOC], act_dt, tag="ob")
                    # DVE PSUM-read CAST costs ~690ns vs ~450ns on scalar;
                    # scalar is exp-saturated during attn3 but idle in drain
                    if self.drain_mode and self.evict_flip:
                        nc.scalar.copy(ob[:], pf[:])
                    else:
                        nc.vector.tensor_copy(ob[:], pf[:])
                    # alternate DMA rings: the sync ring serializes transfers
                    # at ~600ns each and also carries the wt loads — one ring
                    # alone saturates at the ~860ns/unit drain pace
                    # last few units: force the low-completion-latency sync
                    # ring so the end-of-kernel barrier isn't waiting ~2us
                    # on a SWDGE receipt
                    if self.units_done >= NHC * NSQ - 5:
                        dma_eng = nc.sync
                    else:
                        dma_eng = nc.gpsimd if self.evict_flip else nc.sync
                    self.evict_flip = not self.evict_flip
                    dma_eng.dma_start(
                        out_d[sqb * P:(sqb + 1) * P, hc * WOC:(hc + 1) * WOC],
                        ob[:])
                    self.units_done += 1
                    self.unit = None
                else:
                    self.unit[4] = c + 1
                return True

            def drain(self):
                self.drain_mode = True
                while self.emit_mm():
                    pass

        wo = WoEmitter()

        # ---------------- piece 0: k-incremental projections ----------------
        def _compute_piece_kinc(p):
            off, width = PIECES[p]
            packs = hst_tiles.pop(p)
            ps0 = []
            for c in range(NCB):
                pool_c = (psA, psA, psS, psS, psO, psO)[c]
                tag_c = ("a", "a", "s", "s", "po", "po")[c]
                pc = pool_c.tile([P, CW], F32, tag=tag_c, name=f"ps0_{p}_{c}")
                ps0.append(pc)
            for ci, cs in enumerate(((0, 1, 2), (3, 4, 5))):
                for k in range(NKT):
                    for c in cs:
                        nc.tensor.matmul(
                            ps0[c][:, :width], w_all[:, c, k * P:(k + 1) * P],
                            packs[k // KP][:, k % KP, :width],
                            start=(k == 0), stop=(k == NKT - 1))
                if ci == 0:
                    # evict pass-1 groups (q heads: RoPE only, no PE work)
                    # now: frees their PSUM banks ~24us before the next
                    # piece's groups need them
                    _load_hst_piece(1)
                    for c in cs:
                        _finish_block(p, c, ps0[c])
            return ps0

        ps_p0 = _compute_piece_kinc(0)
        # v (c=5) first: its eviction+PE-transposes go to the head of the
        # scalar queue instead of waiting behind the RoPE evictions
        for c in (5, 3, 4):
            _finish_block(0, c, ps_p0[c])

        # ---------------- pieces 1..3 with attn chunk p-1 woven in ----------
        # A chunk's scores all land within its piece, but its pv/rowsum TAIL
        # spills into the next piece's slots (carried in `live`): the piece
        # boundary never stalls the PE waiting on exp chains.
        SLOTS = NCB * 8  # sub-bursts of 4 proj matmuls each
        live = []  # chunks with pending pvs from earlier pieces
        slot = 0
        for p in range(1, len(PIECES)):
            if p + 1 < len(PIECES):
                _load_hst_piece(p + 1)
            packs = hst_tiles.pop(p)
            width = PIECES[p][1]
            at = AttnChunk(p - 1)
            nblocks = len(at.blocks)
            slot0 = slot
            # v group first (see above). In the last piece, also pull k (c=4)
            # forward: attn chunk 3's first scores follow immediately at the
            # region-B boundary and need kT chunk 3 RoPE'd in time.
            corder = (5, 4, 0, 1, 2, 3) if p == len(PIECES) - 1 else (5, 0, 1, 2, 3, 4)
            for c in corder:
                ps = psA.tile([P, CW], F32, tag="a")
                for sb in range(8):
                    for k in range(sb * KP, (sb + 1) * KP):
                        nc.tensor.matmul(
                            ps[:, :width], w_all[:, c, k * P:(k + 1) * P],
                            packs[k // KP][:, k % KP, :width],
                            start=(k == 0), stop=(k == NKT - 1))
                    slot += 1
                    for old in live:
                        old.emit_due_pvs(slot, lag=2)
                    live = [o for o in live if o.pending]
                    target = (slot - slot0) * nblocks // SLOTS
                    while at.s_cursor < min(target, nblocks):
                        at.emit_scores(slot)
                    at.emit_due_pvs(slot, lag=2)
                _finish_block(p, c, ps)
            if at.pending:
                live.append(at)

        # ---------------- region B: attn chunk 3 with wo fillers ------------
        wo.add_chunk(0)
        wo.add_chunk(1)
        # chunk 2 becomes wo-ready only once its carried-over tail drains
        if not live:
            wo.add_chunk(2)
        wo.prefetch(0)
        wo.prefetch(1)
        at3 = AttnChunk(3)
        # pe-ns bookkeeping at ~2GHz: matmul of width W ~ W/2 ns
        pe_ns = 0.0
        filler_debt = 0.0
        LAGNS = 1000.0
        for i in range(len(at3.blocks)):
            _, b = at3.blocks[i]
            t, o = at3._osl(b)
            wblk = CW - o
            at3.emit_scores(pe_ns)
            pe_ns += wblk / 2.0
            # drain the chunk-2 tail carried over from piece 3 (its exps
            # finished during the final proj groups)
            if i >= 1 and live:
                for old in live:
                    for _ in range(2):
                        if old.pending:
                            old.emit_pv()
                            pe_ns += 256.0
                live = [o for o in live if o.pending]
                if not live:
                    wo.add_chunk(2)
            # exp costs ~2 cyc/elem @1.2GHz = wblk*1.667ns; PE side s+pv = wblk ns
            filler_debt += wblk * 1.667 - 2.0 * (wblk / 2.0)
            while filler_debt > 0 and i >= 1:
                if not wo.emit_mm():
                    filler_debt = 0.0
                    break
                pe_ns += WOC / 2.0
                filler_debt -= WOC / 2.0
            # release pvs whose exp is surely done (~LAGNS of PE work ago)
            while at3.pending and pe_ns - at3.pending[0][0] >= LAGNS:
                at3.emit_pv()
                pe_ns += wblk / 2.0
        # tail: release remaining pvs, filling the exp-latency with wo matmuls
        while at3.pending:
            if pe_ns - at3.pending[0][0] >= LAGNS:
                at3.emit_pv()
                pe_ns += 220.0
            elif wo.emit_mm():
                pe_ns += WOC / 2.0
            else:
                at3.emit_pv()
                pe_ns += 220.0
        wo.add_chunk(3)
        wo.drain()

    nc.compile()
    return nc


def _rope_tables():
    inv_freq = (1.0 / (BASE ** (np.arange(0, D, 2, dtype=np.float32) / D))).astype(np.float32)
    pos = np.arange(S, dtype=np.float32)[:, None]
    ang = pos * inv_freq[None, :]              # [S, D/2]
    sin = np.sin(ang).astype(np.float32).T     # [D/2, S]
    cos = np.cos(ang).astype(np.float32).T
    cosT = np.empty((D, S), np.float32)
    cosT[0::2] = cos
    cosT[1::2] = cos
    ssinT = np.empty((D, S), np.float32)
    ssinT[0::2] = -sin
    ssinT[1::2] = sin
    return cosT, ssinT


def _diag_masks():
    p = np.arange(P)[:, None, None]
    t = np.arange(CW // P)[None, :, None]
    f = np.arange(CW)[None, None, :]
    return (f >= P * t + p).astype(np.float32)


_NC_CACHE = {}


def kernel(hidden_states, wq, wk, wv, wo):
    global LAST_RESULT
    act_np = ml_dtypes.bfloat16
    key = "bf16"
    if key not in _NC_CACHE:
        _NC_CACHE[key] = _build()
    nc = _NC_CACHE[key]

    hs = np.asarray(hidden_states, np.float32).reshape(S, HID)
    hsT = np.ascontiguousarray(hs.T).astype(act_np)
    # bf16 tables: halves their DMA and keeps the RoPE muls all-bf16 on DVE
    cosT, ssinT = (t.astype(act_np) for t in _rope_tables())
    dmask = _diag_masks().astype(act_np)

    in_maps = []
    for i in range(NCORES):
        wqkv = np.concatenate(
            [np.asarray(wq, np.float32)[:, i * HL * D:(i + 1) * HL * D],
             np.asarray(wk, np.float32)[:, i * D:(i + 1) * D],
             np.asarray(wv, np.float32)[:, i * D:(i + 1) * D]], axis=1)
        wqkv = np.ascontiguousarray(
            wqkv.reshape(NKT, P, NCB, P).transpose(2, 1, 0, 3).reshape(NCB, P, NKT * P)
        ).astype(act_np)
        wo_i = np.ascontiguousarray(
            np.asarray(wo, np.float32)[i * HL * D:(i + 1) * HL * D, :]
            .reshape(HL, P, NHC, WOC)).astype(act_np)
        in_maps.append({
            "hsT": hsT, "wqkv": wqkv, "wo": wo_i,
            "cosT": cosT, "ssinT": ssinT, "dmask": dmask,
        })

    trace = bool(os.environ.get("BASS_KERNEL_TRACE"))
    res = run_bass_kernel_spmd(nc, in_maps, list(range(NCORES)),
                               trace=trace, trace_cores=[0] if trace else None)
    LAST_RESULT = res
    acc = np.zeros((S, HID), np.float32)
    for i in range(NCORES):
        acc += np.asarray(res.results[i]["out"], np.float32)
    return acc.reshape(B, S, HID)
